# revision 12
# baseline (speedup 1.0000x reference)
"""GQA attention kernel for 8 trn2 NeuronCores (tensor-parallel over heads).

Problem: B=1, S=2048, D=2048, NQ=32 q heads, NKV=8 kv heads, HD=64.
Core i handles q heads 4i..4i+3 and kv head i; out = sum of per-core partials.

v2: all-bf16 matmuls (1 cycle/row vs 4 for fp32 on the PE), x pre-transposed
on the host (kills 256 on-device PE transposes), proj+RMSNorm+RoPE fused per
512-column chunk, ACT stays on the exp table for the whole attention phase,
reciprocals via the fast custom-DVE op, psum->sbuf copies on the Pool engine,
out-projection matmuls interleaved into the attention stream to keep the PE
fed while ACT works through the exps.

Layout (all transposed, zero on-device transposes):
  xT   [128, 16, 2048] bf16  built on host: xT[p, kc, s] = x[s, 128*kc+p]
  Q^T  [128 = 2 heads x 64, S] per head-pair  (lhsT = Wq slice as stored)
  K^T  [64, S] normed+roped, duplicated into partitions 64..127
  V    [128 seq, 16 blocks, 64+1] with a ones column (softmax denominators
       fall out of the PV matmul as row 64)
  S^T block = K^T_slice.T @ Q^T -> exp on ACT -> PV: V_ext.T @ expS^T
  out-proj: lhsT = O^T directly, partial written to DRAM in bf16

RMSNorm over the head dim (= partitions) via ones-selector matmuls; the
per-head g vector is folded into the rstd-broadcast selector on the host.
"""

import os
import sys

sys.path.insert(0, "/opt/trn_rl_repo")

import numpy as np

try:
    import ml_dtypes

    BF = ml_dtypes.bfloat16
except ImportError:  # pragma: no cover
    BF = np.float32

S = 2048
D = 2048
HD = 64
NQ = 32
NKV = 8
P = 128
EPS = 1e-6
SCALE = 0.125  # 1/sqrt(HD)
N_CORES = 8

_CACHE = {}
LAST_RESULTS = None


def _build_nc():
    import concourse.bass as bass
    import concourse.tile as tile
    from concourse import bacc, mybir

    f32 = mybir.dt.float32
    bf16 = mybir.dt.bfloat16
    nc = bacc.Bacc("TRN2", target_bir_lowering=False, debug=False)

    def dram_in(name, shape, dt):
        return nc.dram_tensor(name, list(shape), dt, kind="ExternalInput").ap()

    io = {
        "xt": dram_in("xt", (P, 16, S), bf16),
        "wqa": dram_in("wqa", (P, 16, P), bf16),
        "wqb": dram_in("wqb", (P, 16, P), bf16),
        "wkv": dram_in("wkv", (P, 16, P), bf16),
        "wo": dram_in("wo", (P, 2, D), bf16),
        "cos4": dram_in("cos4", (P, S), bf16),
        "sin4s": dram_in("sin4s", (P, S), bf16),
        "tri": dram_in("tri", (P, P), bf16),
        "ones2": dram_in("ones2", (P, 2), bf16),
        "onesk": dram_in("onesk", (HD, 1), bf16),
        "sel2g": dram_in("sel2g", (2, P), bf16),
        "selk": dram_in("selk", (1, HD), bf16),
        "sel1": dram_in("sel1", (1, HD), bf16),
        "rot2": dram_in("rot2", (P, P), bf16),
        "identb": dram_in("identb", (P, HD), bf16),
        "out": nc.dram_tensor("out", [S, D], bf16, kind="ExternalOutput").ap(),
    }

    from contextlib import ExitStack

    with tile.TileContext(nc) as tc, ExitStack() as ctx:
        _emit(ctx, tc, io, bass, mybir)
    nc.compile()
    return nc


def _emit(ctx, tc, io, bass, mybir):
    nc = tc.nc
    f32 = mybir.dt.float32
    bf16 = mybir.dt.bfloat16
    Exp = mybir.ActivationFunctionType.Exp
    Sqrt = mybir.ActivationFunctionType.Sqrt
    Square = mybir.ActivationFunctionType.Square
    mult = mybir.AluOpType.mult

    cpool = ctx.enter_context(tc.tile_pool(name="consts", bufs=1))
    pers = ctx.enter_context(tc.tile_pool(name="persist", bufs=1))

    # ---- constants / weights into SBUF (DMA order = need order) ----
    def cload(name, shape, dt=bf16):
        t = cpool.tile(list(shape), dt, tag=name, name=name)
        nc.sync.dma_start(t[:], io[name][:])
        return t

    wqa = cload("wqa", (P, 16, P))
    wqb = cload("wqb", (P, 16, P))
    wkv = cload("wkv", (P, 16, P))
    ones2 = cload("ones2", (P, 2))
    onesk = cload("onesk", (HD, 1))
    sel2g = cload("sel2g", (2, P))
    selk = cload("selk", (1, HD))
    sel1 = cload("sel1", (1, HD))
    rot2 = cload("rot2", (P, P))
    identb = cload("identb", (P, HD))
    cos4 = cload("cos4", (P, S))
    sin4s = cload("sin4s", (P, S))
    tri = cload("tri", (P, P))
    wo = cload("wo", (P, 2, D))

    # ---- persistent activations ----
    QT = [pers.tile([P, S], bf16, tag=f"qt{t}", name=f"QT{t}") for t in range(2)]
    KT = pers.tile([P, S], bf16, tag="kt")  # rows 64-127 = copy of rows 0-63
    V = pers.tile([P, 16, HD + 1], bf16, tag="v")
    OT = pers.tile([P, 2, S], bf16, tag="ot")

    nc.vector.memset(V[:, :, HD : HD + 1], 1.0)
    epsc = pers.tile([P, 1], f32, tag="epsc")
    nc.vector.memset(epsc[:], EPS)

    # ---- pools (PSUM: mmp 2 + opp 2 + pop 2 + dpp 2 = 8 banks) ----
    mmp = ctx.enter_context(tc.tile_pool(name="mmp", bufs=2, space="PSUM"))
    opp = ctx.enter_context(tc.tile_pool(name="opp", bufs=2, space="PSUM"))
    pop = ctx.enter_context(tc.tile_pool(name="pop", bufs=2, space="PSUM"))
    dpp = ctx.enter_context(tc.tile_pool(name="dpp", bufs=2, space="PSUM"))

    xp = ctx.enter_context(tc.tile_pool(name="xp", bufs=2))
    sqp = ctx.enter_context(tc.tile_pool(name="sqp", bufs=2))
    tsp = ctx.enter_context(tc.tile_pool(name="tsp", bufs=3))
    stdp = ctx.enter_context(tc.tile_pool(name="stdp", bufs=2))
    rstdp = ctx.enter_context(tc.tile_pool(name="rstdp", bufs=2))
    rsbp = ctx.enter_context(tc.tile_pool(name="rsbp", bufs=2))
    tcp = ctx.enter_context(tc.tile_pool(name="tcp", bufs=2))
    bcbp = ctx.enter_context(tc.tile_pool(name="bcbp", bufs=2))
    esp = ctx.enter_context(tc.tile_pool(name="esp", bufs=3))
    recp = ctx.enter_context(tc.tile_pool(name="recp", bufs=2))
    rebp = ctx.enter_context(tc.tile_pool(name="rebp", bufs=2))
    bcsp = ctx.enter_context(tc.tile_pool(name="bcsp", bufs=2))
    stgp = ctx.enter_context(tc.tile_pool(name="stgp", bufs=2))
    ovp = ctx.enter_context(tc.tile_pool(name="ovp", bufs=2))

    # ================= projection + RMSNorm + RoPE, per 512-col chunk ======
    def proj_chunk(sc):
        cs = slice(sc * 512, (sc + 1) * 512)
        xc = xp.tile([P, 16, 512], bf16, tag="xc", name="xc")
        nc.sync.dma_start(xc[:], io["xt"][:, :, cs])

        def proj(w, pool, tag):
            ps = pool.tile([P, 512], f32, tag=tag, name="ps")
            for kc in range(16):
                nc.tensor.matmul(
                    ps, w[:, kc, :], xc[:, kc, :],
                    start=(kc == 0), stop=(kc == 15),
                )
            # single ACT copy drains the psum bank early; everything
            # downstream (squares, norm multiplies, rope) reads SBUF
            ts = tsp.tile([P, 512], bf16, tag="ts", name="ts")
            nc.scalar.copy(ts, ps)
            return ts

        def norm_rope(ts, m, nh, sumsel, bcsel, T):
            # ts: [>=m, 512] sbuf bf16 pre-norm; T: SBUF bf16 dest [m, 512]
            sq = sqp.tile([P, 512], bf16, tag="sq", name="sq")[:m]
            nc.scalar.activation(sq, ts[:m], Square)
            ssps = dpp.tile([P, 512], f32, tag="dp", name="ssps")[:nh]
            nc.tensor.matmul(ssps, sumsel, sq, start=True, stop=True)
            std = stdp.tile([2, 512], f32, tag="std", name="std")[:nh]
            nc.scalar.activation(std, ssps, Sqrt, bias=epsc[:nh], scale=1.0 / HD)
            rstd = rstdp.tile([2, 512], f32, tag="rstd", name="rstd")[:nh]
            nc.vector.reciprocal_approx_fast(rstd, std)
            rstdb = rsbp.tile([2, 512], bf16, tag="rstdb", name="rstdb")[:nh]
            nc.vector.tensor_copy(rstdb, rstd)
            bc = dpp.tile([P, 512], f32, tag="dp", name="bc")[:m]
            nc.tensor.matmul(bc, bcsel, rstdb, start=True, stop=True)
            # T = ts * bcast(g * rstd)   (g is folded into bcsel on the host)
            nc.vector.tensor_mul(T, ts[:m], bc)
            # RoPE in place on T
            tmpc = tcp.tile([P, 512], bf16, tag="tc", name="tmpc")[:m]
            nc.vector.tensor_mul(tmpc, T, cos4[:m, cs])
            sw = dpp.tile([P, 512], f32, tag="dp", name="sw")[:m]
            nc.tensor.matmul(sw, rot2[:m, :m], T, start=True, stop=True)
            nc.vector.tensor_mul(T, sw, sin4s[:m, cs])
            nc.vector.tensor_add(T, T, tmpc)

        # combined K^T|V^T projection: psum rows 0:64 = K^T, 64:128 = V^T
        tkv = proj(wkv, mmp, "mmp")
        # V^T -> V via PE transposes (bf16 passthrough, out dtype = in dtype)
        for b in range(4):
            ptf = dpp.tile([P, 512], f32, tag="dp", name="ptf")[:, 0:32]
            pt = ptf.bitcast(bf16)
            nc.tensor.transpose(pt, tkv[HD:P, b * P : (b + 1) * P], identb[HD:P, :])
            nc.scalar.copy(V[:, sc * 4 + b, 0:HD], pt)
        norm_rope(tkv, HD, 1, onesk[:, :], selk[:, :], KT[0:HD, cs])
        nc.sync.dma_start(KT[HD:P, cs], KT[0:HD, cs])

        ts0 = proj(wqa, opp, "opp")
        norm_rope(ts0, P, 2, ones2[:, :], sel2g[:, :], QT[0][:, cs])
        ts1 = proj(wqb, mmp, "mmp")
        norm_rope(ts1, P, 2, ones2[:, :], sel2g[:, :], QT[1][:, cs])

    # ================= attention + interleaved out-projection ==============
    def outproj_unit(qc, ms, dc):
        sl = slice(qc * 512 + ms * P, qc * 512 + (ms + 1) * P)
        pso = opp.tile([P, 512], f32, tag="opp", name="pso")
        for kc in range(2):
            nc.tensor.matmul(
                pso, OT[:, kc, sl], wo[:, kc, dc * 512 : (dc + 1) * 512],
                start=(kc == 0), stop=(kc == 1),
            )
        ov = ovp.tile([P, 512], bf16, tag="ov", name="ov")
        # alternate the psum->sbuf drain between DVE and ACT
        if (ms + dc) % 2:
            nc.scalar.copy(ov[:], pso[:])
        else:
            nc.vector.tensor_copy(ov[:], pso[:])
        nc.sync.dma_start(io["out"][sl, dc * 512 : (dc + 1) * 512], ov[:])

    def attn_chunk(qc, pending):
        # pending: list of (ms, dc) outproj units of chunk qc-1 to interleave
        qs = slice(qc * 512, (qc + 1) * 512)
        nkb = 4 * qc + 4
        stride = max(1, (nkb * 4) // 16)  # kb slots per interleaved unit
        slot = 0

        def tick():
            nonlocal slot
            slot += 1
            if pending and slot % stride == 0:
                outproj_unit(*pending.pop(0))

        for h in range(4):
            pair, poff = h // 2, (h % 2) * HD
            Q = QT[pair]
            po = pop.tile([HD + 1, 512], f32, tag="po", name="po")

            def score_exp(kb):
                # diagonal blocks (o >= 0): only columns >= 128*o can attend
                # to this key block -> narrow the score/exp/PV to [co:512]
                o = kb - 4 * qc
                co = max(0, o) * P
                ps = mmp.tile([P, 512], f32, tag="mmp", name="ps")
                nc.tensor.matmul(
                    ps[:, co:512],
                    KT[poff : poff + HD, kb * P : (kb + 1) * P],
                    Q[poff : poff + HD, qc * 512 + co : (qc + 1) * 512],
                    start=True, stop=True,
                )
                es = esp.tile([P, 512], bf16, tag="es", name="es")
                nc.scalar.activation(es[:, co:512], ps[:, co:512], Exp, scale=SCALE)
                if o >= 0:
                    # triangular mask on the 128-col diagonal sub-block
                    nc.vector.tensor_mul(es[:, co : co + P], es[:, co : co + P], tri)
                return es, co

            def pv(kb, es, co):
                nc.tensor.matmul(
                    po[:, co:512], V[:, kb, :], es[:, co:512],
                    start=(kb == 0), stop=(kb == nkb - 1),
                    skip_group_check=True,
                )

            prev, pco = score_exp(0)
            for kb in range(1, nkb):
                cur, cco = score_exp(kb)
                pv(kb - 1, prev, pco)
                tick()
                prev, pco = cur, cco
            pv(nkb - 1, prev, pco)
            tick()

            # normalize: row HD of po holds the softmax denominators
            den = recp.tile([1, 512], f32, tag="den", name="den")
            nc.scalar.copy(den, po[HD : HD + 1, :])
            rec = recp.tile([1, 512], f32, tag="rec", name="rec")
            nc.vector.reciprocal_approx_fast(rec, den)
            recb = rebp.tile([1, 512], bf16, tag="recb", name="recb")
            nc.vector.tensor_copy(recb, rec)
            bca = dpp.tile([P, 512], f32, tag="dp", name="bca")[:HD]
            nc.tensor.matmul(bca, sel1[:, :], recb, start=True, stop=True)
            bcs = bcsp.tile([HD, 512], bf16, tag="bcs", name="bcs")
            nc.scalar.copy(bcs, bca)
            if poff == 0:
                nc.vector.tensor_mul(OT[0:HD, pair, qs], po[0:HD, :], bcs)
            else:
                stg = stgp.tile([HD, 512], bf16, tag="stg", name="stg")
                nc.vector.tensor_mul(stg, po[0:HD, :], bcs)
                nc.sync.dma_start(OT[HD:P, pair, qs], stg[:])

        while pending:
            outproj_unit(*pending.pop(0))

    for sc in range(4):
        proj_chunk(sc)
    units = [(ms, dc) for ms in range(4) for dc in range(4)]
    for qc in range(4):
        attn_chunk(qc, [(qc - 1, ms, dc) for (ms, dc) in units] if qc else [])
    for ms, dc in units:
        outproj_unit(3, ms, dc)


def _prep_core_inputs(i, x, cos, sin, g_q, g_k, Wq, Wk, Wv, Wo):
    c0 = i * 4 * HD
    k0 = i * HD

    def b(a):
        return np.ascontiguousarray(a).astype(BF)

    x2 = x.reshape(S, D)
    xt = x2.reshape(S, 16, P).transpose(2, 1, 0)  # [p, kc, s]
    wqa = Wq[:, c0 : c0 + P].reshape(16, P, P).transpose(1, 0, 2)
    wqb = Wq[:, c0 + P : c0 + 2 * P].reshape(16, P, P).transpose(1, 0, 2)
    wkv = np.concatenate(
        [Wk[:, k0 : k0 + HD], Wv[:, k0 : k0 + HD]], axis=1
    ).reshape(16, P, P).transpose(1, 0, 2)
    wo = Wo[c0 : c0 + 2 * P, :].reshape(2, P, D).transpose(1, 0, 2)
    cosT = cos.T.astype(np.float32)  # [32, S]
    sinT = sin.T.astype(np.float32)
    cos4 = np.tile(cosT, (4, 1))
    sin4s = np.concatenate([-sinT, sinT, -sinT, sinT], axis=0)
    tri = np.triu(np.ones((P, P), dtype=np.float32))  # [k within blk, q within blk]
    ones2 = np.zeros((P, 2), dtype=np.float32)
    ones2[:HD, 0] = 1.0
    ones2[HD:, 1] = 1.0
    sel2g = np.zeros((2, P), dtype=np.float32)
    sel2g[0, :HD] = g_q
    sel2g[1, HD:] = g_q
    r64 = np.roll(np.eye(HD, dtype=np.float32), 32, axis=0)
    rot2 = np.zeros((P, P), dtype=np.float32)
    rot2[:HD, :HD] = r64
    rot2[HD:, HD:] = r64
    return {
        "xt": b(xt),
        "wqa": b(wqa), "wqb": b(wqb), "wkv": b(wkv), "wo": b(wo),
        "cos4": b(cos4), "sin4s": b(sin4s), "tri": b(tri),
        "ones2": b(ones2),
        "onesk": b(np.ones((HD, 1), dtype=np.float32)),
        "sel2g": b(sel2g),
        "selk": b(np.asarray(g_k, dtype=np.float32).reshape(1, HD)),
        "sel1": b(np.ones((1, HD), dtype=np.float32)),
        "rot2": b(rot2),
        "identb": b(np.concatenate([np.eye(HD), np.eye(HD)], axis=0)),
    }


def kernel(x, cos, sin, g_q, g_k, Wq, Wk, Wv, Wo):
    global LAST_RESULTS
    from concourse.bass_utils import run_bass_kernel_spmd

    if "nc" not in _CACHE:
        _CACHE["nc"] = _build_nc()
    nc = _CACHE["nc"]

    args = [np.asarray(a, dtype=np.float32) for a in
            (x, cos, sin, g_q, g_k, Wq, Wk, Wv, Wo)]
    in_maps = [_prep_core_inputs(i, *args) for i in range(N_CORES)]
    trace = bool(os.environ.get("BASS_TRACE"))
    res = run_bass_kernel_spmd(nc, in_maps, list(range(N_CORES)), trace=trace)
    LAST_RESULTS = res
    out = np.zeros((S, D), dtype=np.float32)
    for r in res.results:
        out += np.asarray(r["out"], dtype=np.float32)
    return out.reshape(1, S, D)


# revision 16
# speedup vs baseline: 1.0651x; 1.0651x over previous
"""GQA attention kernel for 8 trn2 NeuronCores (tensor-parallel over heads).

Problem: B=1, S=2048, D=2048, NQ=32 q heads, NKV=8 kv heads, HD=64.
Core i handles q heads 4i..4i+3 and kv head i; out = sum of per-core partials.

v2: all-bf16 matmuls (1 cycle/row vs 4 for fp32 on the PE), x pre-transposed
on the host (kills 256 on-device PE transposes), proj+RMSNorm+RoPE fused per
512-column chunk, ACT stays on the exp table for the whole attention phase,
reciprocals via the fast custom-DVE op, psum->sbuf copies on the Pool engine,
out-projection matmuls interleaved into the attention stream to keep the PE
fed while ACT works through the exps.

Layout (all transposed, zero on-device transposes):
  xT   [128, 16, 2048] bf16  built on host: xT[p, kc, s] = x[s, 128*kc+p]
  Q^T  [128 = 2 heads x 64, S] per head-pair  (lhsT = Wq slice as stored)
  K^T  [64, S] normed+roped, duplicated into partitions 64..127
  V    [128 seq, 16 blocks, 64+1] with a ones column (softmax denominators
       fall out of the PV matmul as row 64)
  S^T block = K^T_slice.T @ Q^T -> exp on ACT -> PV: V_ext.T @ expS^T
  out-proj: lhsT = O^T directly, partial written to DRAM in bf16

RMSNorm over the head dim (= partitions) via ones-selector matmuls; the
per-head g vector is folded into the rstd-broadcast selector on the host.
"""

import os
import sys

sys.path.insert(0, "/opt/trn_rl_repo")

import numpy as np

try:
    import ml_dtypes

    BF = ml_dtypes.bfloat16
except ImportError:  # pragma: no cover
    BF = np.float32

S = 2048
D = 2048
HD = 64
NQ = 32
NKV = 8
P = 128
EPS = 1e-6
SCALE = 0.125  # 1/sqrt(HD)
N_CORES = 8

_CACHE = {}
LAST_RESULTS = None


def _build_nc():
    import concourse.bass as bass
    import concourse.tile as tile
    from concourse import bacc, mybir

    f32 = mybir.dt.float32
    bf16 = mybir.dt.bfloat16
    nc = bacc.Bacc("TRN2", target_bir_lowering=False, debug=False)

    def dram_in(name, shape, dt):
        return nc.dram_tensor(name, list(shape), dt, kind="ExternalInput").ap()

    io = {
        "xt": dram_in("xt", (P, 16, S), bf16),
        "wqa": dram_in("wqa", (P, 16, P), bf16),
        "wqb": dram_in("wqb", (P, 16, P), bf16),
        "wkv": dram_in("wkv", (P, 16, P), bf16),
        "wo": dram_in("wo", (P, 2, D), bf16),
        "cos4": dram_in("cos4", (P, S), bf16),
        "sin4s": dram_in("sin4s", (P, S), bf16),
        "tri": dram_in("tri", (P, P), bf16),
        "ones2": dram_in("ones2", (P, 2), bf16),
        "onesk": dram_in("onesk", (HD, 1), bf16),
        "sel66": dram_in("sel66", (66, P), bf16),
        "sel1": dram_in("sel1", (1, HD), bf16),
        "rot2": dram_in("rot2", (P, P), bf16),
        "identb": dram_in("identb", (P, HD), bf16),
        "out": nc.dram_tensor("out", [S, D], bf16, kind="ExternalOutput").ap(),
    }

    from contextlib import ExitStack

    with tile.TileContext(nc) as tc, ExitStack() as ctx:
        _emit(ctx, tc, io, bass, mybir)
    nc.compile()
    return nc


def _emit(ctx, tc, io, bass, mybir):
    import math
    from collections import deque

    nc = tc.nc
    f32 = mybir.dt.float32
    bf16 = mybir.dt.bfloat16
    Exp = mybir.ActivationFunctionType.Exp
    Sqrt = mybir.ActivationFunctionType.Sqrt
    mult = mybir.AluOpType.mult

    cpool = ctx.enter_context(tc.tile_pool(name="consts", bufs=1))
    pers = ctx.enter_context(tc.tile_pool(name="persist", bufs=1))

    # ---- constants / weights into SBUF (DMA order = need order) ----
    def cload(name, shape, dt=bf16):
        t = cpool.tile(list(shape), dt, tag=name, name=name)
        nc.sync.dma_start(t[:], io[name][:])
        return t

    wkv = cload("wkv", (P, 16, P))
    wqa = cload("wqa", (P, 16, P))
    wqb = cload("wqb", (P, 16, P))
    ones2 = cload("ones2", (P, 2))
    onesk = cload("onesk", (HD, 1))
    sel66 = cload("sel66", (66, P))
    sel1 = cload("sel1", (1, HD))
    rot2 = cload("rot2", (P, P))
    identb = cload("identb", (P, HD))
    cos4 = cload("cos4", (P, S))
    sin4s = cload("sin4s", (P, S))
    tri = cload("tri", (P, P))
    wo = cload("wo", (P, 2, D))

    # ---- persistent activations ----
    QT = [pers.tile([P, S], bf16, tag=f"qt{t}", name=f"QT{t}") for t in range(2)]
    KT = pers.tile([P, S], bf16, tag="kt")  # rows 64-127 = copy of rows 0-63
    V = pers.tile([P, 16, HD + 1], bf16, tag="v")
    OT = pers.tile([P, 2, S], bf16, tag="ot")

    nc.vector.memset(V[:, :, HD : HD + 1], 1.0)
    epsc = pers.tile([P, 1], f32, tag="epsc")
    nc.vector.memset(epsc[:], EPS)

    # ---- pools (PSUM: mmp 2 + opp 2 + pop 2 + dpp 2 = 8 banks) ----
    mmp = ctx.enter_context(tc.tile_pool(name="mmp", bufs=2, space="PSUM"))
    opp = ctx.enter_context(tc.tile_pool(name="opp", bufs=2, space="PSUM"))
    pop = ctx.enter_context(tc.tile_pool(name="pop", bufs=2, space="PSUM"))
    dpp = ctx.enter_context(tc.tile_pool(name="dpp", bufs=2, space="PSUM"))

    xp = ctx.enter_context(tc.tile_pool(name="xp", bufs=3))
    sqp = ctx.enter_context(tc.tile_pool(name="sqp", bufs=2))
    tsp = ctx.enter_context(tc.tile_pool(name="tsp", bufs=3))
    stdp = ctx.enter_context(tc.tile_pool(name="stdp", bufs=2))
    rstdp = ctx.enter_context(tc.tile_pool(name="rstdp", bufs=2))
    rsbp = ctx.enter_context(tc.tile_pool(name="rsbp", bufs=2))
    tcp = ctx.enter_context(tc.tile_pool(name="tcp", bufs=2))
    esp = ctx.enter_context(tc.tile_pool(name="esp", bufs=3))
    recp = ctx.enter_context(tc.tile_pool(name="recp", bufs=2))
    rebp = ctx.enter_context(tc.tile_pool(name="rebp", bufs=2))
    bcsp = ctx.enter_context(tc.tile_pool(name="bcsp", bufs=2))
    stgp = ctx.enter_context(tc.tile_pool(name="stgp", bufs=2))
    ovp = ctx.enter_context(tc.tile_pool(name="ovp", bufs=2))

    # ========== proj + RMSNorm + RoPE for one 512-col chunk, as closures ====
    # Emitted either eagerly or interleaved into the attention stream via the
    # filler deque, so the PE never starves while ACT digests exps.
    def proj_closures(sc):
        cs = slice(sc * 512, (sc + 1) * 512)
        units = []
        u = units.append
        state = {}

        def dma_piece(k4):
            def f():
                if "xc" not in state:
                    state["xc"] = xp.tile([P, 16, 512], bf16, tag="xc", name="xc")
                nc.sync.dma_start(
                    state["xc"][:, k4 * 4 : (k4 + 1) * 4, :],
                    io["xt"][:, k4 * 4 : (k4 + 1) * 4, cs],
                )
            return f

        for k4 in range(4):
            u(dma_piece(k4))

        def mm_pair(w, pstag, pool, kc):
            def f():
                if pstag not in state:
                    state[pstag] = pool.tile([P, 512], f32, tag=pool.name, name=pstag)
                ps = state[pstag]
                for k in (kc, kc + 1):
                    nc.tensor.matmul(
                        ps, w[:, k, :], state["xc"][:, k, :],
                        start=(k == 0), stop=(k == 15),
                    )
            return f

        def ts_copy(pstag, tstag):
            def f():
                state[tstag] = tsp.tile([P, 512], bf16, tag="ts", name=tstag)
                nc.scalar.copy(state[tstag], state[pstag])
            return f

        def sq_stats(pstag, tstag, m, rowbase, sumsel):
            def f():
                if "stats" not in state:
                    state["stats"] = dpp.tile([P, 512], f32, tag="dp", name="stats")
                    # rows 2:32, 34:64, 65 are never written by the stat
                    # matmuls but are read by the bundled Sqrt; engine
                    # partition bases must be 32-aligned, so define the whole
                    # tile and let the stat matmuls overlay their rows
                    nc.vector.memset(state["stats"][:, :], 1.0)
                sq = sqp.tile([P, 512], bf16, tag="sq", name="sq")[:m]
                nc.vector.tensor_mul(sq, state[pstag][:m], state[tstag][:m])
                nh = 1 if m == HD else 2
                nc.tensor.matmul(
                    state["stats"][rowbase : rowbase + nh], sumsel, sq,
                    start=True, stop=True, skip_group_check=True,
                )
            return f

        # combined K^T|V^T projection: psum rows 0:64 = K^T, 64:128 = V^T
        for kc in range(0, 16, 2):
            u(mm_pair(wkv, "pskv", mmp, kc))
        u(ts_copy("pskv", "tkv"))

        def vtrans(b):
            def f():
                ptf = dpp.tile([P, 512], f32, tag="dp", name="ptf")[:, 0:32]
                pt = ptf.bitcast(bf16)
                nc.tensor.transpose(
                    pt, state["tkv"][HD:P, b * P : (b + 1) * P], identb[HD:P, :]
                )
                nc.scalar.copy(V[:, sc * 4 + b, 0:HD], pt)
            return f

        for b in range(4):
            u(vtrans(b))
        u(sq_stats("pskv", "tkv", HD, 64, onesk[:, :]))

        for kc in range(0, 16, 2):
            u(mm_pair(wqa, "ps0", opp, kc))
        u(ts_copy("ps0", "ts0"))
        u(sq_stats("ps0", "ts0", P, 0, ones2[:, :]))
        for kc in range(0, 16, 2):
            u(mm_pair(wqb, "ps1", mmp, kc))
        u(ts_copy("ps1", "ts1"))
        u(sq_stats("ps1", "ts1", P, 32, ones2[:, :]))

        def stats_fin():
            # one Sqrt / reciprocal / cast for all 5 head rows of the chunk
            stdall = stdp.tile([66, 512], f32, tag="std", name="stdall")
            nc.scalar.activation(
                stdall, state["stats"][0:66], Sqrt, bias=epsc[:66], scale=1.0 / HD
            )
            rstd = rstdp.tile([66, 512], f32, tag="rstd", name="rstd")
            nc.vector.reciprocal_approx_fast(rstd, stdall)
            state["rsb"] = rsbp.tile([66, 512], bf16, tag="rstdb", name="rsb")
            nc.vector.tensor_copy(state["rsb"], rstd)
        u(stats_fin)

        def bc_tmul(tstag, m, rowbase, T):
            def f():
                bc = dpp.tile([P, 512], f32, tag="dp", name="bc")[:m]
                nh = 1 if m == HD else 2
                nc.tensor.matmul(
                    bc, sel66[rowbase : rowbase + nh, :m],
                    state["rsb"][rowbase : rowbase + nh],
                    start=True, stop=True,
                )
                # T = ts * bcast(g * rstd)  (g folded into sel66 on the host)
                nc.vector.tensor_mul(T, state[tstag][:m], bc)
            return f

        def rope_a(m, T):
            def f():
                state["tmpc"] = tcp.tile([P, 512], bf16, tag="tc", name="tmpc")[:m]
                nc.vector.tensor_mul(state["tmpc"], T, cos4[:m, cs])
                sw = dpp.tile([P, 512], f32, tag="dp", name="sw")[:m]
                nc.tensor.matmul(sw, rot2[:m, :m], T, start=True, stop=True)
                state["sw"] = sw
            return f

        def rope_b(m, T, kdup=False):
            def f():
                nc.vector.tensor_mul(T, state["sw"], sin4s[:m, cs])
                nc.vector.tensor_add(T, T, state["tmpc"])
                if kdup:
                    nc.sync.dma_start(KT[HD:P, cs], KT[0:HD, cs])
            return f

        u(bc_tmul("tkv", HD, 64, KT[0:HD, cs]))
        u(rope_a(HD, KT[0:HD, cs]))
        u(rope_b(HD, KT[0:HD, cs], kdup=True))
        u(bc_tmul("ts0", P, 0, QT[0][:, cs]))
        u(rope_a(P, QT[0][:, cs]))
        u(rope_b(P, QT[0][:, cs]))
        u(bc_tmul("ts1", P, 32, QT[1][:, cs]))
        u(rope_a(P, QT[1][:, cs]))
        u(rope_b(P, QT[1][:, cs]))
        return units

    def outproj_units(qc):
        units = []
        for ms in range(4):
            for dc in range(4):
                def f(ms=ms, dc=dc):
                    sl = slice(qc * 512 + ms * P, qc * 512 + (ms + 1) * P)
                    pso = opp.tile([P, 512], f32, tag="opp", name="pso")
                    for kc in range(2):
                        nc.tensor.matmul(
                            pso, OT[:, kc, sl],
                            wo[:, kc, dc * 512 : (dc + 1) * 512],
                            start=(kc == 0), stop=(kc == 1),
                        )
                    ov = ovp.tile([P, 512], bf16, tag="ov", name="ov")
                    if (ms + dc) % 2:
                        nc.scalar.copy(ov[:], pso[:])
                    else:
                        nc.vector.tensor_copy(ov[:], pso[:])
                    nc.sync.dma_start(
                        io["out"][sl, dc * 512 : (dc + 1) * 512], ov[:]
                    )
                units.append(f)
        return units

    # ================= attention, with filler interleave ===================
    fill = deque()

    def attn_chunk(qc):
        qs = slice(qc * 512, (qc + 1) * 512)
        nkb = 4 * qc + 4
        slots = [4 * nkb]

        def tick():
            k = math.ceil(len(fill) / slots[0]) if fill else 0
            slots[0] -= 1
            for _ in range(min(k, len(fill))):
                fill.popleft()()

        for h in range(4):
            pair, poff = h // 2, (h % 2) * HD
            Q = QT[pair]
            po = pop.tile([HD + 1, 512], f32, tag="po", name="po")

            def score_exp(kb):
                # diagonal blocks (o >= 0): only columns >= 128*o can attend
                # to this key block -> narrow the score/exp/PV to [co:512]
                o = kb - 4 * qc
                co = max(0, o) * P
                ps = mmp.tile([P, 512], f32, tag="mmp", name="ps")
                nc.tensor.matmul(
                    ps[:, co:512],
                    KT[poff : poff + HD, kb * P : (kb + 1) * P],
                    Q[poff : poff + HD, qc * 512 + co : (qc + 1) * 512],
                    start=True, stop=True,
                )
                es = esp.tile([P, 512], bf16, tag="es", name="es")
                nc.scalar.activation(es[:, co:512], ps[:, co:512], Exp, scale=SCALE)
                if o >= 0:
                    # triangular mask on the 128-col diagonal sub-block
                    nc.vector.tensor_mul(es[:, co : co + P], es[:, co : co + P], tri)
                return es, co

            def pv(kb, es, co):
                nc.tensor.matmul(
                    po[:, co:512], V[:, kb, :], es[:, co:512],
                    start=(kb == 0), stop=(kb == nkb - 1),
                    skip_group_check=True,
                )

            prev, pco = score_exp(0)
            for kb in range(1, nkb):
                cur, cco = score_exp(kb)
                pv(kb - 1, prev, pco)
                tick()
                prev, pco = cur, cco
            pv(nkb - 1, prev, pco)
            tick()

            # normalize: row HD of po holds the softmax denominators
            den = recp.tile([1, 512], f32, tag="den", name="den")
            nc.vector.tensor_copy(den, po[HD : HD + 1, :])
            rec = recp.tile([1, 512], f32, tag="rec", name="rec")
            nc.vector.reciprocal_approx_fast(rec, den)
            recb = rebp.tile([1, 512], bf16, tag="recb", name="recb")
            nc.vector.tensor_copy(recb, rec)
            bca = dpp.tile([P, 512], f32, tag="dp", name="bca")[:HD]
            nc.tensor.matmul(bca, sel1[:, :], recb, start=True, stop=True)
            bcs = bcsp.tile([HD, 512], bf16, tag="bcs", name="bcs")
            nc.vector.tensor_copy(bcs, bca)
            if poff == 0:
                nc.vector.tensor_mul(OT[0:HD, pair, qs], po[0:HD, :], bcs)
            else:
                stg = stgp.tile([HD, 512], bf16, tag="stg", name="stg")
                nc.vector.tensor_mul(stg, po[0:HD, :], bcs)
                nc.sync.dma_start(OT[HD:P, pair, qs], stg[:])

    for f in proj_closures(0):
        f()
    for f in proj_closures(1):
        f()
    for qc in range(4):
        if qc + 2 <= 3:
            fill.extend(proj_closures(qc + 2))
        attn_chunk(qc)
        fill.extend(outproj_units(qc))
    while fill:
        fill.popleft()()


def _prep_core_inputs(i, x, cos, sin, g_q, g_k, Wq, Wk, Wv, Wo):
    c0 = i * 4 * HD
    k0 = i * HD

    def b(a):
        return np.ascontiguousarray(a).astype(BF)

    x2 = x.reshape(S, D)
    xt = x2.reshape(S, 16, P).transpose(2, 1, 0)  # [p, kc, s]
    wqa = Wq[:, c0 : c0 + P].reshape(16, P, P).transpose(1, 0, 2)
    wqb = Wq[:, c0 + P : c0 + 2 * P].reshape(16, P, P).transpose(1, 0, 2)
    wkv = np.concatenate(
        [Wk[:, k0 : k0 + HD], Wv[:, k0 : k0 + HD]], axis=1
    ).reshape(16, P, P).transpose(1, 0, 2)
    wo = Wo[c0 : c0 + 2 * P, :].reshape(2, P, D).transpose(1, 0, 2)
    cosT = cos.T.astype(np.float32)  # [32, S]
    sinT = sin.T.astype(np.float32)
    cos4 = np.tile(cosT, (4, 1))
    sin4s = np.concatenate([-sinT, sinT, -sinT, sinT], axis=0)
    tri = np.triu(np.ones((P, P), dtype=np.float32))  # [k within blk, q within blk]
    ones2 = np.zeros((P, 2), dtype=np.float32)
    ones2[:HD, 0] = 1.0
    ones2[HD:, 1] = 1.0
    sel66 = np.zeros((66, P), dtype=np.float32)
    for rb in (0, 32):
        sel66[rb, :HD] = g_q
        sel66[rb + 1, HD:] = g_q
    sel66[64, :HD] = g_k
    r64 = np.roll(np.eye(HD, dtype=np.float32), 32, axis=0)
    rot2 = np.zeros((P, P), dtype=np.float32)
    rot2[:HD, :HD] = r64
    rot2[HD:, HD:] = r64
    return {
        "xt": b(xt),
        "wqa": b(wqa), "wqb": b(wqb), "wkv": b(wkv), "wo": b(wo),
        "cos4": b(cos4), "sin4s": b(sin4s), "tri": b(tri),
        "ones2": b(ones2),
        "onesk": b(np.ones((HD, 1), dtype=np.float32)),
        "sel66": b(sel66),
        "sel1": b(np.ones((1, HD), dtype=np.float32)),
        "rot2": b(rot2),
        "identb": b(np.concatenate([np.eye(HD), np.eye(HD)], axis=0)),
    }


def kernel(x, cos, sin, g_q, g_k, Wq, Wk, Wv, Wo):
    global LAST_RESULTS
    from concourse.bass_utils import run_bass_kernel_spmd

    if "nc" not in _CACHE:
        _CACHE["nc"] = _build_nc()
    nc = _CACHE["nc"]

    args = [np.asarray(a, dtype=np.float32) for a in
            (x, cos, sin, g_q, g_k, Wq, Wk, Wv, Wo)]
    in_maps = [_prep_core_inputs(i, *args) for i in range(N_CORES)]
    trace = bool(os.environ.get("BASS_TRACE"))
    res = run_bass_kernel_spmd(nc, in_maps, list(range(N_CORES)), trace=trace)
    LAST_RESULTS = res
    out = np.zeros((S, D), dtype=np.float32)
    for r in res.results:
        out += np.asarray(r["out"], dtype=np.float32)
    return out.reshape(1, S, D)


# revision 17
# speedup vs baseline: 1.0673x; 1.0021x over previous
"""GQA attention kernel for 8 trn2 NeuronCores (tensor-parallel over heads).

Problem: B=1, S=2048, D=2048, NQ=32 q heads, NKV=8 kv heads, HD=64.
Core i handles q heads 4i..4i+3 and kv head i; out = sum of per-core partials.

v2: all-bf16 matmuls (1 cycle/row vs 4 for fp32 on the PE), x pre-transposed
on the host (kills 256 on-device PE transposes), proj+RMSNorm+RoPE fused per
512-column chunk, ACT stays on the exp table for the whole attention phase,
reciprocals via the fast custom-DVE op, psum->sbuf copies on the Pool engine,
out-projection matmuls interleaved into the attention stream to keep the PE
fed while ACT works through the exps.

Layout (all transposed, zero on-device transposes):
  xT   [128, 16, 2048] bf16  built on host: xT[p, kc, s] = x[s, 128*kc+p]
  Q^T  [128 = 2 heads x 64, S] per head-pair  (lhsT = Wq slice as stored)
  K^T  [64, S] normed+roped, duplicated into partitions 64..127
  V    [128 seq, 16 blocks, 64+1] with a ones column (softmax denominators
       fall out of the PV matmul as row 64)
  S^T block = K^T_slice.T @ Q^T -> exp on ACT -> PV: V_ext.T @ expS^T
  out-proj: lhsT = O^T directly, partial written to DRAM in bf16

RMSNorm over the head dim (= partitions) via ones-selector matmuls; the
per-head g vector is folded into the rstd-broadcast selector on the host.
"""

import os
import sys

sys.path.insert(0, "/opt/trn_rl_repo")

import numpy as np

try:
    import ml_dtypes

    BF = ml_dtypes.bfloat16
except ImportError:  # pragma: no cover
    BF = np.float32

S = 2048
D = 2048
HD = 64
NQ = 32
NKV = 8
P = 128
EPS = 1e-6
SCALE = 0.125  # 1/sqrt(HD)
N_CORES = 8

_CACHE = {}
LAST_RESULTS = None


def _build_nc():
    import concourse.bass as bass
    import concourse.tile as tile
    from concourse import bacc, mybir

    f32 = mybir.dt.float32
    bf16 = mybir.dt.bfloat16
    nc = bacc.Bacc("TRN2", target_bir_lowering=False, debug=False)

    def dram_in(name, shape, dt):
        return nc.dram_tensor(name, list(shape), dt, kind="ExternalInput").ap()

    io = {
        "xt": dram_in("xt", (P, 16, S), bf16),
        "wqa": dram_in("wqa", (P, 16, P), bf16),
        "wqb": dram_in("wqb", (P, 16, P), bf16),
        "wkv": dram_in("wkv", (P, 16, P), bf16),
        "wo": dram_in("wo", (P, 2, D), bf16),
        "cos4": dram_in("cos4", (P, S), bf16),
        "sin4s": dram_in("sin4s", (P, S), bf16),
        "tri": dram_in("tri", (P, P), bf16),
        "ones2": dram_in("ones2", (P, 2), bf16),
        "onesk": dram_in("onesk", (HD, 1), bf16),
        "sel66": dram_in("sel66", (66, P), bf16),
        "sel1": dram_in("sel1", (1, HD), bf16),
        "rot2": dram_in("rot2", (P, P), bf16),
        "identb": dram_in("identb", (P, HD), bf16),
        "out": nc.dram_tensor("out", [S, D], bf16, kind="ExternalOutput").ap(),
    }

    from contextlib import ExitStack

    with tile.TileContext(nc) as tc, ExitStack() as ctx:
        _emit(ctx, tc, io, bass, mybir)
    nc.compile()
    return nc


def _emit(ctx, tc, io, bass, mybir):
    import math
    from collections import deque

    nc = tc.nc
    f32 = mybir.dt.float32
    bf16 = mybir.dt.bfloat16
    Exp = mybir.ActivationFunctionType.Exp
    Sqrt = mybir.ActivationFunctionType.Sqrt
    mult = mybir.AluOpType.mult

    cpool = ctx.enter_context(tc.tile_pool(name="consts", bufs=1))
    pers = ctx.enter_context(tc.tile_pool(name="persist", bufs=1))

    # ---- constants / weights into SBUF (DMA order = need order) ----
    def cload(name, shape, dt=bf16):
        t = cpool.tile(list(shape), dt, tag=name, name=name)
        nc.sync.dma_start(t[:], io[name][:])
        return t

    wkv = cload("wkv", (P, 16, P))
    wqa = cload("wqa", (P, 16, P))
    wqb = cload("wqb", (P, 16, P))
    ones2 = cload("ones2", (P, 2))
    onesk = cload("onesk", (HD, 1))
    sel66 = cload("sel66", (66, P))
    sel1 = cload("sel1", (1, HD))
    rot2 = cload("rot2", (P, P))
    identb = cload("identb", (P, HD))
    cos4 = cload("cos4", (P, S))
    sin4s = cload("sin4s", (P, S))
    tri = cload("tri", (P, P))
    wo = cload("wo", (P, 2, D))

    # ---- persistent activations ----
    QT = [pers.tile([P, S], bf16, tag=f"qt{t}", name=f"QT{t}") for t in range(2)]
    KT = pers.tile([P, S], bf16, tag="kt")  # rows 64-127 = copy of rows 0-63
    V = pers.tile([P, 16, HD + 1], bf16, tag="v")
    OT = pers.tile([P, 2, S], bf16, tag="ot")

    nc.vector.memset(V[:, :, HD : HD + 1], 1.0)
    epsc = pers.tile([P, 1], f32, tag="epsc")
    nc.vector.memset(epsc[:], EPS)

    # ---- pools (PSUM: mmp 2 + opp 2 + pop 2 + dpp 2 = 8 banks) ----
    mmp = ctx.enter_context(tc.tile_pool(name="mmp", bufs=2, space="PSUM"))
    opp = ctx.enter_context(tc.tile_pool(name="opp", bufs=2, space="PSUM"))
    pop = ctx.enter_context(tc.tile_pool(name="pop", bufs=2, space="PSUM"))
    dpp = ctx.enter_context(tc.tile_pool(name="dpp", bufs=2, space="PSUM"))

    xp = ctx.enter_context(tc.tile_pool(name="xp", bufs=3))
    sqp = ctx.enter_context(tc.tile_pool(name="sqp", bufs=2))
    tsp = ctx.enter_context(tc.tile_pool(name="tsp", bufs=3))
    stdp = ctx.enter_context(tc.tile_pool(name="stdp", bufs=2))
    rstdp = ctx.enter_context(tc.tile_pool(name="rstdp", bufs=2))
    rsbp = ctx.enter_context(tc.tile_pool(name="rsbp", bufs=2))
    tcp = ctx.enter_context(tc.tile_pool(name="tcp", bufs=2))
    esp = ctx.enter_context(tc.tile_pool(name="esp", bufs=4))
    recp = ctx.enter_context(tc.tile_pool(name="recp", bufs=2))
    rebp = ctx.enter_context(tc.tile_pool(name="rebp", bufs=2))
    bcsp = ctx.enter_context(tc.tile_pool(name="bcsp", bufs=2))
    stgp = ctx.enter_context(tc.tile_pool(name="stgp", bufs=2))
    ovp = ctx.enter_context(tc.tile_pool(name="ovp", bufs=4))

    # ========== proj + RMSNorm + RoPE for one 512-col chunk, as closures ====
    # Emitted either eagerly or interleaved into the attention stream via the
    # filler deque, so the PE never starves while ACT digests exps.
    def proj_closures(sc):
        cs = slice(sc * 512, (sc + 1) * 512)
        units = []
        u = units.append
        state = {}

        def dma_piece(k4):
            def f():
                if "xc" not in state:
                    state["xc"] = xp.tile([P, 16, 512], bf16, tag="xc", name="xc")
                nc.sync.dma_start(
                    state["xc"][:, k4 * 4 : (k4 + 1) * 4, :],
                    io["xt"][:, k4 * 4 : (k4 + 1) * 4, cs],
                )
            return f

        for k4 in range(4):
            u(dma_piece(k4))

        def mm_pair(w, pstag, pool, kc):
            def f():
                if pstag not in state:
                    state[pstag] = pool.tile([P, 512], f32, tag=pool.name, name=pstag)
                ps = state[pstag]
                for k in (kc, kc + 1):
                    nc.tensor.matmul(
                        ps, w[:, k, :], state["xc"][:, k, :],
                        start=(k == 0), stop=(k == 15),
                    )
            return f

        def ts_copy(pstag, tstag):
            def f():
                state[tstag] = tsp.tile([P, 512], bf16, tag="ts", name=tstag)
                nc.scalar.copy(state[tstag], state[pstag])
            return f

        def sq_stats(pstag, tstag, m, rowbase, sumsel):
            def f():
                if "stats" not in state:
                    state["stats"] = dpp.tile([P, 512], f32, tag="dp", name="stats")
                    # rows 2:32, 34:64, 65 are never written by the stat
                    # matmuls but are read by the bundled Sqrt; engine
                    # partition bases must be 32-aligned, so define the whole
                    # tile and let the stat matmuls overlay their rows
                    nc.vector.memset(state["stats"][:, :], 1.0)
                sq = sqp.tile([P, 512], bf16, tag="sq", name="sq")[:m]
                nc.vector.tensor_mul(sq, state[pstag][:m], state[tstag][:m])
                nh = 1 if m == HD else 2
                nc.tensor.matmul(
                    state["stats"][rowbase : rowbase + nh], sumsel, sq,
                    start=True, stop=True, skip_group_check=True,
                )
            return f

        # combined K^T|V^T projection: psum rows 0:64 = K^T, 64:128 = V^T
        for kc in range(0, 16, 2):
            u(mm_pair(wkv, "pskv", mmp, kc))
        u(ts_copy("pskv", "tkv"))

        def vtrans(b):
            def f():
                ptf = dpp.tile([P, 512], f32, tag="dp", name="ptf")[:, 0:32]
                pt = ptf.bitcast(bf16)
                nc.tensor.transpose(
                    pt, state["tkv"][HD:P, b * P : (b + 1) * P], identb[HD:P, :]
                )
                nc.scalar.copy(V[:, sc * 4 + b, 0:HD], pt)
            return f

        for b in range(4):
            u(vtrans(b))
        u(sq_stats("pskv", "tkv", HD, 64, onesk[:, :]))

        for kc in range(0, 16, 2):
            u(mm_pair(wqa, "ps0", opp, kc))
        u(ts_copy("ps0", "ts0"))
        u(sq_stats("ps0", "ts0", P, 0, ones2[:, :]))
        for kc in range(0, 16, 2):
            u(mm_pair(wqb, "ps1", mmp, kc))
        u(ts_copy("ps1", "ts1"))
        u(sq_stats("ps1", "ts1", P, 32, ones2[:, :]))

        def stats_fin():
            # one Sqrt / reciprocal / cast for all 5 head rows of the chunk
            stdall = stdp.tile([66, 512], f32, tag="std", name="stdall")
            nc.scalar.activation(
                stdall, state["stats"][0:66], Sqrt, bias=epsc[:66], scale=1.0 / HD
            )
            rstd = rstdp.tile([66, 512], f32, tag="rstd", name="rstd")
            nc.vector.reciprocal_approx_fast(rstd, stdall)
            state["rsb"] = rsbp.tile([66, 512], bf16, tag="rstdb", name="rsb")
            nc.vector.tensor_copy(state["rsb"], rstd)
        u(stats_fin)

        def bc_tmul(tstag, m, rowbase, T):
            def f():
                bc = dpp.tile([P, 512], f32, tag="dp", name="bc")[:m]
                nh = 1 if m == HD else 2
                nc.tensor.matmul(
                    bc, sel66[rowbase : rowbase + nh, :m],
                    state["rsb"][rowbase : rowbase + nh],
                    start=True, stop=True,
                )
                # T = ts * bcast(g * rstd)  (g folded into sel66 on the host)
                nc.vector.tensor_mul(T, state[tstag][:m], bc)
            return f

        def rope_a(m, T):
            def f():
                state["tmpc"] = tcp.tile([P, 512], bf16, tag="tc", name="tmpc")[:m]
                nc.vector.tensor_mul(state["tmpc"], T, cos4[:m, cs])
                sw = dpp.tile([P, 512], f32, tag="dp", name="sw")[:m]
                nc.tensor.matmul(sw, rot2[:m, :m], T, start=True, stop=True)
                state["sw"] = sw
            return f

        def rope_b(m, T, kdup=False):
            def f():
                nc.vector.tensor_mul(T, state["sw"], sin4s[:m, cs])
                nc.vector.tensor_add(T, T, state["tmpc"])
                if kdup:
                    nc.sync.dma_start(KT[HD:P, cs], KT[0:HD, cs])
            return f

        u(bc_tmul("tkv", HD, 64, KT[0:HD, cs]))
        u(rope_a(HD, KT[0:HD, cs]))
        u(rope_b(HD, KT[0:HD, cs], kdup=True))
        u(bc_tmul("ts0", P, 0, QT[0][:, cs]))
        u(rope_a(P, QT[0][:, cs]))
        u(rope_b(P, QT[0][:, cs]))
        u(bc_tmul("ts1", P, 32, QT[1][:, cs]))
        u(rope_a(P, QT[1][:, cs]))
        u(rope_b(P, QT[1][:, cs]))
        return units

    def outproj_units(qc):
        units = []
        for ms in range(4):
            for dc in range(4):
                def f(ms=ms, dc=dc):
                    sl = slice(qc * 512 + ms * P, qc * 512 + (ms + 1) * P)
                    pso = opp.tile([P, 512], f32, tag="opp", name="pso")
                    for kc in range(2):
                        nc.tensor.matmul(
                            pso, OT[:, kc, sl],
                            wo[:, kc, dc * 512 : (dc + 1) * 512],
                            start=(kc == 0), stop=(kc == 1),
                        )
                    ov = ovp.tile([P, 512], bf16, tag="ov", name="ov")
                    if (ms + dc) % 2:
                        nc.scalar.copy(ov[:], pso[:])
                    else:
                        nc.vector.tensor_copy(ov[:], pso[:])
                    nc.gpsimd.dma_start(
                        io["out"][sl, dc * 512 : (dc + 1) * 512], ov[:]
                    )
                units.append(f)
        return units

    # ================= attention, with filler interleave ===================
    fill = deque()

    def attn_chunk(qc):
        qs = slice(qc * 512, (qc + 1) * 512)
        nkb = 4 * qc + 4
        slots = [4 * nkb]

        def tick():
            k = math.ceil(len(fill) / slots[0]) if fill else 0
            slots[0] -= 1
            for _ in range(min(k, len(fill))):
                fill.popleft()()

        for h in range(4):
            pair, poff = h // 2, (h % 2) * HD
            Q = QT[pair]
            po = pop.tile([HD + 1, 512], f32, tag="po", name="po")

            def score_exp(kb):
                # diagonal blocks (o >= 0): only columns >= 128*o can attend
                # to this key block -> narrow the score/exp/PV to [co:512]
                o = kb - 4 * qc
                co = max(0, o) * P
                ps = mmp.tile([P, 512], f32, tag="mmp", name="ps")
                nc.tensor.matmul(
                    ps[:, co:512],
                    KT[poff : poff + HD, kb * P : (kb + 1) * P],
                    Q[poff : poff + HD, qc * 512 + co : (qc + 1) * 512],
                    start=True, stop=True,
                )
                es = esp.tile([P, 512], bf16, tag="es", name="es")
                nc.scalar.activation(es[:, co:512], ps[:, co:512], Exp, scale=SCALE)
                if o >= 0:
                    # triangular mask on the 128-col diagonal sub-block
                    nc.gpsimd.tensor_mul(es[:, co : co + P], es[:, co : co + P], tri)
                return es, co

            def pv(kb, es, co):
                nc.tensor.matmul(
                    po[:, co:512], V[:, kb, :], es[:, co:512],
                    start=(kb == 0), stop=(kb == nkb - 1),
                    skip_group_check=True,
                )

            prev, pco = score_exp(0)
            for kb in range(1, nkb):
                cur, cco = score_exp(kb)
                pv(kb - 1, prev, pco)
                tick()
                prev, pco = cur, cco
            pv(nkb - 1, prev, pco)
            tick()

            # normalize: row HD of po holds the softmax denominators
            den = recp.tile([1, 512], f32, tag="den", name="den")
            nc.vector.tensor_copy(den, po[HD : HD + 1, :])
            rec = recp.tile([1, 512], f32, tag="rec", name="rec")
            nc.vector.reciprocal_approx_fast(rec, den)
            recb = rebp.tile([1, 512], bf16, tag="recb", name="recb")
            nc.vector.tensor_copy(recb, rec)
            bca = dpp.tile([P, 512], f32, tag="dp", name="bca")[:HD]
            nc.tensor.matmul(bca, sel1[:, :], recb, start=True, stop=True)
            bcs = bcsp.tile([HD, 512], bf16, tag="bcs", name="bcs")
            nc.vector.tensor_copy(bcs, bca)
            if poff == 0:
                nc.vector.tensor_mul(OT[0:HD, pair, qs], po[0:HD, :], bcs)
            else:
                stg = stgp.tile([HD, 512], bf16, tag="stg", name="stg")
                nc.vector.tensor_mul(stg, po[0:HD, :], bcs)
                nc.sync.dma_start(OT[HD:P, pair, qs], stg[:])

    for f in proj_closures(0):
        f()
    for f in proj_closures(1):
        f()
    for qc in range(4):
        if qc + 2 <= 3:
            fill.extend(proj_closures(qc + 2))
        attn_chunk(qc)
        fill.extend(outproj_units(qc))
    while fill:
        fill.popleft()()


def _prep_core_inputs(i, x, cos, sin, g_q, g_k, Wq, Wk, Wv, Wo):
    c0 = i * 4 * HD
    k0 = i * HD

    def b(a):
        return np.ascontiguousarray(a).astype(BF)

    x2 = x.reshape(S, D)
    xt = x2.reshape(S, 16, P).transpose(2, 1, 0)  # [p, kc, s]
    wqa = Wq[:, c0 : c0 + P].reshape(16, P, P).transpose(1, 0, 2)
    wqb = Wq[:, c0 + P : c0 + 2 * P].reshape(16, P, P).transpose(1, 0, 2)
    wkv = np.concatenate(
        [Wk[:, k0 : k0 + HD], Wv[:, k0 : k0 + HD]], axis=1
    ).reshape(16, P, P).transpose(1, 0, 2)
    wo = Wo[c0 : c0 + 2 * P, :].reshape(2, P, D).transpose(1, 0, 2)
    cosT = cos.T.astype(np.float32)  # [32, S]
    sinT = sin.T.astype(np.float32)
    cos4 = np.tile(cosT, (4, 1))
    sin4s = np.concatenate([-sinT, sinT, -sinT, sinT], axis=0)
    tri = np.triu(np.ones((P, P), dtype=np.float32))  # [k within blk, q within blk]
    ones2 = np.zeros((P, 2), dtype=np.float32)
    ones2[:HD, 0] = 1.0
    ones2[HD:, 1] = 1.0
    sel66 = np.zeros((66, P), dtype=np.float32)
    for rb in (0, 32):
        sel66[rb, :HD] = g_q
        sel66[rb + 1, HD:] = g_q
    sel66[64, :HD] = g_k
    r64 = np.roll(np.eye(HD, dtype=np.float32), 32, axis=0)
    rot2 = np.zeros((P, P), dtype=np.float32)
    rot2[:HD, :HD] = r64
    rot2[HD:, HD:] = r64
    return {
        "xt": b(xt),
        "wqa": b(wqa), "wqb": b(wqb), "wkv": b(wkv), "wo": b(wo),
        "cos4": b(cos4), "sin4s": b(sin4s), "tri": b(tri),
        "ones2": b(ones2),
        "onesk": b(np.ones((HD, 1), dtype=np.float32)),
        "sel66": b(sel66),
        "sel1": b(np.ones((1, HD), dtype=np.float32)),
        "rot2": b(rot2),
        "identb": b(np.concatenate([np.eye(HD), np.eye(HD)], axis=0)),
    }


def kernel(x, cos, sin, g_q, g_k, Wq, Wk, Wv, Wo):
    global LAST_RESULTS
    from concourse.bass_utils import run_bass_kernel_spmd

    if "nc" not in _CACHE:
        _CACHE["nc"] = _build_nc()
    nc = _CACHE["nc"]

    args = [np.asarray(a, dtype=np.float32) for a in
            (x, cos, sin, g_q, g_k, Wq, Wk, Wv, Wo)]
    in_maps = [_prep_core_inputs(i, *args) for i in range(N_CORES)]
    trace = bool(os.environ.get("BASS_TRACE"))
    res = run_bass_kernel_spmd(nc, in_maps, list(range(N_CORES)), trace=trace)
    LAST_RESULTS = res
    out = np.zeros((S, D), dtype=np.float32)
    for r in res.results:
        out += np.asarray(r["out"], dtype=np.float32)
    return out.reshape(1, S, D)


# revision 19
# speedup vs baseline: 1.0956x; 1.0265x over previous
"""GQA attention kernel for 8 trn2 NeuronCores (tensor-parallel over heads).

Problem: B=1, S=2048, D=2048, NQ=32 q heads, NKV=8 kv heads, HD=64.
Core i handles q heads 4i..4i+3 and kv head i; out = sum of per-core partials.

v2: all-bf16 matmuls (1 cycle/row vs 4 for fp32 on the PE), x pre-transposed
on the host (kills 256 on-device PE transposes), proj+RMSNorm+RoPE fused per
512-column chunk, ACT stays on the exp table for the whole attention phase,
reciprocals via the fast custom-DVE op, psum->sbuf copies on the Pool engine,
out-projection matmuls interleaved into the attention stream to keep the PE
fed while ACT works through the exps.

Layout (all transposed, zero on-device transposes):
  xT   [128, 16, 2048] bf16  built on host: xT[p, kc, s] = x[s, 128*kc+p]
  Q^T  [128 = 2 heads x 64, S] per head-pair  (lhsT = Wq slice as stored)
  K^T  [64, S] normed+roped, duplicated into partitions 64..127
  V    [128 seq, 16 blocks, 64+1] with a ones column (softmax denominators
       fall out of the PV matmul as row 64)
  S^T block = K^T_slice.T @ Q^T -> exp on ACT -> PV: V_ext.T @ expS^T
  out-proj: lhsT = O^T directly, partial written to DRAM in bf16

RMSNorm over the head dim (= partitions) via ones-selector matmuls; the
per-head g vector is folded into the rstd-broadcast selector on the host.
"""

import os
import sys

sys.path.insert(0, "/opt/trn_rl_repo")

import numpy as np

try:
    import ml_dtypes

    BF = ml_dtypes.bfloat16
except ImportError:  # pragma: no cover
    BF = np.float32

S = 2048
D = 2048
HD = 64
NQ = 32
NKV = 8
P = 128
EPS = 1e-6
SCALE = 0.125  # 1/sqrt(HD)
N_CORES = 8

_CACHE = {}
LAST_RESULTS = None


def _build_nc():
    import concourse.bass as bass
    import concourse.tile as tile
    from concourse import bacc, mybir

    f32 = mybir.dt.float32
    bf16 = mybir.dt.bfloat16
    nc = bacc.Bacc("TRN2", target_bir_lowering=False, debug=False)

    def dram_in(name, shape, dt):
        return nc.dram_tensor(name, list(shape), dt, kind="ExternalInput").ap()

    io = {
        "xt": dram_in("xt", (P, 16, S), bf16),
        "wqa": dram_in("wqa", (P, 16, P), bf16),
        "wqb": dram_in("wqb", (P, 16, P), bf16),
        "wkv": dram_in("wkv", (P, 16, P), bf16),
        "wo": dram_in("wo", (P, 2, D), bf16),
        "tabs": dram_in("tabs", (P, 2 * S), bf16),
        "cpack": dram_in("cpack", (P, 528), bf16),
        "out": nc.dram_tensor("out", [S, D], bf16, kind="ExternalOutput").ap(),
    }

    from contextlib import ExitStack

    with tile.TileContext(nc) as tc, ExitStack() as ctx:
        _emit(ctx, tc, io, bass, mybir)
    nc.compile()
    return nc


def _emit(ctx, tc, io, bass, mybir):
    import math
    from collections import deque

    nc = tc.nc
    f32 = mybir.dt.float32
    bf16 = mybir.dt.bfloat16
    Exp = mybir.ActivationFunctionType.Exp
    Sqrt = mybir.ActivationFunctionType.Sqrt
    mult = mybir.AluOpType.mult

    cpool = ctx.enter_context(tc.tile_pool(name="consts", bufs=1))
    pers = ctx.enter_context(tc.tile_pool(name="persist", bufs=1))

    # ---- constants / weights into SBUF (DMA order = need order) ----
    def cload(name, shape, dt=bf16, eng=None):
        t = cpool.tile(list(shape), dt, tag=name, name=name)
        (eng or nc.sync).dma_start(t[:], io[name][:])
        return t

    # split DGE programming across both hardware queues at startup
    wkv = cload("wkv", (P, 16, P))
    cp = cload("cpack", (P, 528))
    wqa = cload("wqa", (P, 16, P), eng=nc.scalar)
    wqb = cload("wqb", (P, 16, P), eng=nc.scalar)
    tabs = cload("tabs", (P, 2 * S), eng=nc.scalar)
    wo = cload("wo", (P, 2, D), eng=nc.scalar)
    rot2 = cp[:, 0:128]
    tri = cp[:, 128:256]
    identb = cp[:, 256:320]
    sel66 = cp[0:66, 320:448]
    ones2 = cp[:, 448:450]
    onesk = cp[0:HD, 450:451]
    sel1 = cp[0:1, 451:515]
    cos4 = tabs[:, 0:S]
    sin4s = tabs[:, S : 2 * S]

    # ---- persistent activations ----
    QT = [pers.tile([P, S], bf16, tag=f"qt{t}", name=f"QT{t}") for t in range(2)]
    KT = pers.tile([P, S], bf16, tag="kt")  # rows 64-127 = copy of rows 0-63
    V = pers.tile([P, 16, HD + 1], bf16, tag="v")
    OT = pers.tile([P, 2, S], bf16, tag="ot")

    nc.vector.memset(V[:, :, HD : HD + 1], 1.0)
    epsc = pers.tile([P, 1], f32, tag="epsc")
    nc.vector.memset(epsc[:], EPS)

    # ---- pools (PSUM: mmp 2 + opp 2 + pop 2 + dpp 2 = 8 banks) ----
    mmp = ctx.enter_context(tc.tile_pool(name="mmp", bufs=2, space="PSUM"))
    opp = ctx.enter_context(tc.tile_pool(name="opp", bufs=2, space="PSUM"))
    pop = ctx.enter_context(tc.tile_pool(name="pop", bufs=2, space="PSUM"))
    dpp = ctx.enter_context(tc.tile_pool(name="dpp", bufs=2, space="PSUM"))

    xp = ctx.enter_context(tc.tile_pool(name="xp", bufs=3))
    sqp = ctx.enter_context(tc.tile_pool(name="sqp", bufs=2))
    tsp = ctx.enter_context(tc.tile_pool(name="tsp", bufs=3))
    stdp = ctx.enter_context(tc.tile_pool(name="stdp", bufs=2))
    rstdp = ctx.enter_context(tc.tile_pool(name="rstdp", bufs=2))
    rsbp = ctx.enter_context(tc.tile_pool(name="rsbp", bufs=2))
    tcp = ctx.enter_context(tc.tile_pool(name="tcp", bufs=2))
    esp = ctx.enter_context(tc.tile_pool(name="esp", bufs=4))
    recp = ctx.enter_context(tc.tile_pool(name="recp", bufs=2))
    rebp = ctx.enter_context(tc.tile_pool(name="rebp", bufs=2))
    bcsp = ctx.enter_context(tc.tile_pool(name="bcsp", bufs=2))
    stgp = ctx.enter_context(tc.tile_pool(name="stgp", bufs=2))
    ovp = ctx.enter_context(tc.tile_pool(name="ovp", bufs=4))

    # ========== proj + RMSNorm + RoPE for one 512-col chunk, as closures ====
    # Emitted either eagerly or interleaved into the attention stream via the
    # filler deque, so the PE never starves while ACT digests exps.
    def proj_closures(sc):
        cs = slice(sc * 512, (sc + 1) * 512)
        units = []
        u = units.append
        state = {}

        def dma_xc():
            state["xc"] = xp.tile([P, 16, 512], bf16, tag="xc", name="xc")
            nc.sync.dma_start(state["xc"][:], io["xt"][:, :, cs])
        u(dma_xc)

        def mm_pair(w, pstag, pool, kc):
            def f():
                if pstag not in state:
                    state[pstag] = pool.tile([P, 512], f32, tag=pool.name, name=pstag)
                ps = state[pstag]
                for k in (kc, kc + 1):
                    nc.tensor.matmul(
                        ps, w[:, k, :], state["xc"][:, k, :],
                        start=(k == 0), stop=(k == 15),
                    )
            return f

        def ts_copy(pstag, tstag):
            def f():
                state[tstag] = tsp.tile([P, 512], bf16, tag="ts", name=tstag)
                nc.scalar.copy(state[tstag], state[pstag])
            return f

        def sq_stats(pstag, tstag, m, rowbase, sumsel):
            def f():
                if "stats" not in state:
                    state["stats"] = dpp.tile([P, 512], f32, tag="dp", name="stats")
                    # rows 2:32, 34:64, 65 are never written by the stat
                    # matmuls but are read by the bundled Sqrt; engine
                    # partition bases must be 32-aligned, so define the whole
                    # tile and let the stat matmuls overlay their rows
                    nc.vector.memset(state["stats"][:, :], 1.0)
                sq = sqp.tile([P, 512], bf16, tag="sq", name="sq")[:m]
                nc.vector.tensor_mul(sq, state[pstag][:m], state[tstag][:m])
                nh = 1 if m == HD else 2
                nc.tensor.matmul(
                    state["stats"][rowbase : rowbase + nh], sumsel, sq,
                    start=True, stop=True, skip_group_check=True,
                )
            return f

        # combined K^T|V^T projection: psum rows 0:64 = K^T, 64:128 = V^T
        for kc in range(0, 16, 2):
            u(mm_pair(wkv, "pskv", mmp, kc))
        u(ts_copy("pskv", "tkv"))

        def vtrans(b):
            def f():
                ptf = dpp.tile([P, 512], f32, tag="dp", name="ptf")[:, 0:32]
                pt = ptf.bitcast(bf16)
                nc.tensor.transpose(
                    pt, state["tkv"][HD:P, b * P : (b + 1) * P], identb[HD:P]
                )
                nc.scalar.copy(V[:, sc * 4 + b, 0:HD], pt)
            return f

        for b in range(4):
            u(vtrans(b))
        u(sq_stats("pskv", "tkv", HD, 64, onesk))

        for kc in range(0, 16, 2):
            u(mm_pair(wqa, "ps0", opp, kc))
        u(ts_copy("ps0", "ts0"))
        u(sq_stats("ps0", "ts0", P, 0, ones2))
        for kc in range(0, 16, 2):
            u(mm_pair(wqb, "ps1", mmp, kc))
        u(ts_copy("ps1", "ts1"))
        u(sq_stats("ps1", "ts1", P, 32, ones2))

        def stats_fin():
            # one Sqrt / reciprocal / cast for all 5 head rows of the chunk
            stdall = stdp.tile([66, 512], f32, tag="std", name="stdall")
            nc.scalar.activation(
                stdall, state["stats"][0:66], Sqrt, bias=epsc[:66], scale=1.0 / HD
            )
            rstd = rstdp.tile([66, 512], f32, tag="rstd", name="rstd")
            nc.vector.reciprocal_approx_fast(rstd, stdall)
            state["rsb"] = rsbp.tile([66, 512], bf16, tag="rstdb", name="rsb")
            nc.vector.tensor_copy(state["rsb"], rstd)
        u(stats_fin)

        def bc_tmul(tstag, m, rowbase, T):
            def f():
                bc = dpp.tile([P, 512], f32, tag="dp", name="bc")[:m]
                nh = 1 if m == HD else 2
                nc.tensor.matmul(
                    bc, sel66[rowbase : rowbase + nh, :m],
                    state["rsb"][rowbase : rowbase + nh],
                    start=True, stop=True,
                )
                # T = ts * bcast(g * rstd)  (g folded into sel66 on the host)
                nc.vector.tensor_mul(T, state[tstag][:m], bc)
            return f

        def rope_a(m, T):
            def f():
                state["tmpc"] = tcp.tile([P, 512], bf16, tag="tc", name="tmpc")[:m]
                nc.vector.tensor_mul(state["tmpc"], T, cos4[:m, cs])
                sw = dpp.tile([P, 512], f32, tag="dp", name="sw")[:m]
                nc.tensor.matmul(sw, rot2[:m, :m], T, start=True, stop=True)
                state["sw"] = sw
            return f

        def rope_b(m, T, kdup=False):
            def f():
                nc.vector.tensor_mul(T, state["sw"], sin4s[:m, cs])
                nc.vector.tensor_add(T, T, state["tmpc"])
                if kdup:
                    nc.sync.dma_start(KT[HD:P, cs], KT[0:HD, cs])
            return f

        u(bc_tmul("tkv", HD, 64, KT[0:HD, cs]))
        u(rope_a(HD, KT[0:HD, cs]))
        u(rope_b(HD, KT[0:HD, cs], kdup=True))
        u(bc_tmul("ts0", P, 0, QT[0][:, cs]))
        u(rope_a(P, QT[0][:, cs]))
        u(rope_b(P, QT[0][:, cs]))
        u(bc_tmul("ts1", P, 32, QT[1][:, cs]))
        u(rope_a(P, QT[1][:, cs]))
        u(rope_b(P, QT[1][:, cs]))
        return units

    def outproj_units(qc):
        units = []
        state = {}
        for ms in range(4):
            for dc in range(4):
                def f(ms=ms, dc=dc):
                    sl = slice(qc * 512 + ms * P, qc * 512 + (ms + 1) * P)
                    pso = opp.tile([P, 512], f32, tag="opp", name="pso")
                    for kc in range(2):
                        nc.tensor.matmul(
                            pso, OT[:, kc, sl],
                            wo[:, kc, dc * 512 : (dc + 1) * 512],
                            start=(kc == 0), stop=(kc == 1),
                        )
                    if dc == 0:
                        state[ms] = ovp.tile([P, 4, 512], bf16, tag="ov", name="ov")
                    ov = state[ms]
                    if (ms + dc) % 2:
                        nc.scalar.copy(ov[:, dc, :], pso[:])
                    else:
                        nc.vector.tensor_copy(ov[:, dc, :], pso[:])
                    if dc == 3:
                        nc.sync.dma_start(io["out"][sl, :], ov[:])
                units.append(f)
        return units

    # ================= attention, with filler interleave ===================
    fill = deque()

    def attn_chunk(qc):
        qs = slice(qc * 512, (qc + 1) * 512)
        nkb = 4 * qc + 4
        slots = [4 * nkb]

        def tick():
            k = math.ceil(len(fill) / slots[0]) if fill else 0
            slots[0] -= 1
            for _ in range(min(k, len(fill))):
                fill.popleft()()

        for h in range(4):
            pair, poff = h // 2, (h % 2) * HD
            Q = QT[pair]
            po = pop.tile([HD + 1, 512], f32, tag="po", name="po")

            def score_exp(kb):
                # diagonal blocks (o >= 0): only columns >= 128*o can attend
                # to this key block -> narrow the score/exp/PV to [co:512]
                o = kb - 4 * qc
                co = max(0, o) * P
                ps = mmp.tile([P, 512], f32, tag="mmp", name="ps")
                nc.tensor.matmul(
                    ps[:, co:512],
                    KT[poff : poff + HD, kb * P : (kb + 1) * P],
                    Q[poff : poff + HD, qc * 512 + co : (qc + 1) * 512],
                    start=True, stop=True,
                )
                es = esp.tile([P, 512], bf16, tag="es", name="es")
                nc.scalar.activation(es[:, co:512], ps[:, co:512], Exp, scale=SCALE)
                if o >= 0:
                    # triangular mask on the 128-col diagonal sub-block
                    nc.gpsimd.tensor_mul(es[:, co : co + P], es[:, co : co + P], tri)
                return es, co

            def pv(kb, es, co):
                nc.tensor.matmul(
                    po[:, co:512], V[:, kb, :], es[:, co:512],
                    start=(kb == 0), stop=(kb == nkb - 1),
                    skip_group_check=True,
                )

            prev, pco = score_exp(0)
            for kb in range(1, nkb):
                cur, cco = score_exp(kb)
                pv(kb - 1, prev, pco)
                tick()
                prev, pco = cur, cco
            pv(nkb - 1, prev, pco)
            tick()

            # normalize: row HD of po holds the softmax denominators
            den = recp.tile([1, 512], f32, tag="den", name="den")
            nc.vector.tensor_copy(den, po[HD : HD + 1, :])
            rec = recp.tile([1, 512], f32, tag="rec", name="rec")
            nc.vector.reciprocal_approx_fast(rec, den)
            recb = rebp.tile([1, 512], bf16, tag="recb", name="recb")
            nc.vector.tensor_copy(recb, rec)
            bca = dpp.tile([P, 512], f32, tag="dp", name="bca")[:HD]
            nc.tensor.matmul(bca, sel1, recb, start=True, stop=True)
            bcs = bcsp.tile([HD, 512], bf16, tag="bcs", name="bcs")
            nc.vector.tensor_copy(bcs, bca)
            if poff == 0:
                nc.vector.tensor_mul(OT[0:HD, pair, qs], po[0:HD, :], bcs)
            else:
                stg = stgp.tile([HD, 512], bf16, tag="stg", name="stg")
                nc.vector.tensor_mul(stg, po[0:HD, :], bcs)
                nc.sync.dma_start(OT[HD:P, pair, qs], stg[:])

    for f in proj_closures(0):
        f()
    for f in proj_closures(1):
        f()
    for qc in range(4):
        if qc + 2 <= 3:
            fill.extend(proj_closures(qc + 2))
        attn_chunk(qc)
        fill.extend(outproj_units(qc))
    while fill:
        fill.popleft()()


def _prep_core_inputs(i, x, cos, sin, g_q, g_k, Wq, Wk, Wv, Wo):
    c0 = i * 4 * HD
    k0 = i * HD

    def b(a):
        return np.ascontiguousarray(a).astype(BF)

    x2 = x.reshape(S, D)
    xt = x2.reshape(S, 16, P).transpose(2, 1, 0)  # [p, kc, s]
    wqa = Wq[:, c0 : c0 + P].reshape(16, P, P).transpose(1, 0, 2)
    wqb = Wq[:, c0 + P : c0 + 2 * P].reshape(16, P, P).transpose(1, 0, 2)
    wkv = np.concatenate(
        [Wk[:, k0 : k0 + HD], Wv[:, k0 : k0 + HD]], axis=1
    ).reshape(16, P, P).transpose(1, 0, 2)
    wo = Wo[c0 : c0 + 2 * P, :].reshape(2, P, D).transpose(1, 0, 2)
    cosT = cos.T.astype(np.float32)  # [32, S]
    sinT = sin.T.astype(np.float32)
    cos4 = np.tile(cosT, (4, 1))
    sin4s = np.concatenate([-sinT, sinT, -sinT, sinT], axis=0)
    tabs = np.concatenate([cos4, sin4s], axis=1)  # [128, 4096]
    tri = np.triu(np.ones((P, P), dtype=np.float32))  # [k within blk, q within blk]
    ones2 = np.zeros((P, 2), dtype=np.float32)
    ones2[:HD, 0] = 1.0
    ones2[HD:, 1] = 1.0
    sel66 = np.zeros((66, P), dtype=np.float32)
    for rb in (0, 32):
        sel66[rb, :HD] = g_q
        sel66[rb + 1, HD:] = g_q
    sel66[64, :HD] = g_k
    r64 = np.roll(np.eye(HD, dtype=np.float32), 32, axis=0)
    rot2 = np.zeros((P, P), dtype=np.float32)
    rot2[:HD, :HD] = r64
    rot2[HD:, HD:] = r64
    cpack = np.zeros((P, 528), dtype=np.float32)
    cpack[:, 0:128] = rot2
    cpack[:, 128:256] = tri
    cpack[:, 256:320] = np.concatenate([np.eye(HD), np.eye(HD)], axis=0)
    cpack[0:66, 320:448] = sel66
    cpack[:, 448:450] = ones2
    cpack[0:HD, 450] = 1.0
    cpack[0, 451:515] = 1.0
    return {
        "xt": b(xt),
        "wqa": b(wqa), "wqb": b(wqb), "wkv": b(wkv), "wo": b(wo),
        "tabs": b(tabs), "cpack": b(cpack),
    }


def kernel(x, cos, sin, g_q, g_k, Wq, Wk, Wv, Wo):
    global LAST_RESULTS
    from concourse.bass_utils import run_bass_kernel_spmd

    if "nc" not in _CACHE:
        _CACHE["nc"] = _build_nc()
    nc = _CACHE["nc"]

    args = [np.asarray(a, dtype=np.float32) for a in
            (x, cos, sin, g_q, g_k, Wq, Wk, Wv, Wo)]
    in_maps = [_prep_core_inputs(i, *args) for i in range(N_CORES)]
    trace = bool(os.environ.get("BASS_TRACE"))
    res = run_bass_kernel_spmd(nc, in_maps, list(range(N_CORES)), trace=trace)
    LAST_RESULTS = res
    out = np.zeros((S, D), dtype=np.float32)
    for r in res.results:
        out += np.asarray(r["out"], dtype=np.float32)
    return out.reshape(1, S, D)


# revision 20
# speedup vs baseline: 1.1740x; 1.0716x over previous
"""GQA attention kernel for 8 trn2 NeuronCores (tensor-parallel over heads).

Problem: B=1, S=2048, D=2048, NQ=32 q heads, NKV=8 kv heads, HD=64.
Core i handles q heads 4i..4i+3 and kv head i; out = sum of per-core partials.

v2: all-bf16 matmuls (1 cycle/row vs 4 for fp32 on the PE), x pre-transposed
on the host (kills 256 on-device PE transposes), proj+RMSNorm+RoPE fused per
512-column chunk, ACT stays on the exp table for the whole attention phase,
reciprocals via the fast custom-DVE op, psum->sbuf copies on the Pool engine,
out-projection matmuls interleaved into the attention stream to keep the PE
fed while ACT works through the exps.

Layout (all transposed, zero on-device transposes):
  xT   [128, 16, 2048] bf16  built on host: xT[p, kc, s] = x[s, 128*kc+p]
  Q^T  [128 = 2 heads x 64, S] per head-pair  (lhsT = Wq slice as stored)
  K^T  [64, S] normed+roped, duplicated into partitions 64..127
  V    [128 seq, 16 blocks, 64+1] with a ones column (softmax denominators
       fall out of the PV matmul as row 64)
  S^T block = K^T_slice.T @ Q^T -> exp on ACT -> PV: V_ext.T @ expS^T
  out-proj: lhsT = O^T directly, partial written to DRAM in bf16

RMSNorm over the head dim (= partitions) via ones-selector matmuls; the
per-head g vector is folded into the rstd-broadcast selector on the host.
"""

import os
import sys

sys.path.insert(0, "/opt/trn_rl_repo")

import numpy as np

try:
    import ml_dtypes

    BF = ml_dtypes.bfloat16
except ImportError:  # pragma: no cover
    BF = np.float32

S = 2048
D = 2048
HD = 64
NQ = 32
NKV = 8
P = 128
EPS = 1e-6
SCALE = 0.125  # 1/sqrt(HD)
N_CORES = 8

_CACHE = {}
LAST_RESULTS = None


def _build_nc():
    import concourse.bass as bass
    import concourse.tile as tile
    from concourse import bacc, mybir

    f32 = mybir.dt.float32
    bf16 = mybir.dt.bfloat16
    nc = bacc.Bacc("TRN2", target_bir_lowering=False, debug=False)

    def dram_in(name, shape, dt):
        return nc.dram_tensor(name, list(shape), dt, kind="ExternalInput").ap()

    io = {
        "xt": dram_in("xt", (P, 16, S), bf16),
        "wqa": dram_in("wqa", (P, 16, P), bf16),
        "wqb": dram_in("wqb", (P, 16, P), bf16),
        "wkv": dram_in("wkv", (P, 16, P), bf16),
        "wo": dram_in("wo", (P, 2, D), bf16),
        "tabs": dram_in("tabs", (P, 2 * S), bf16),
        "cpack": dram_in("cpack", (P, 528), bf16),
        "out": nc.dram_tensor("out", [S, D], bf16, kind="ExternalOutput").ap(),
    }

    from contextlib import ExitStack

    with tile.TileContext(nc) as tc, ExitStack() as ctx:
        _emit(ctx, tc, io, bass, mybir)
    nc.compile()
    return nc


def _emit(ctx, tc, io, bass, mybir):
    import math
    from collections import deque

    nc = tc.nc
    f32 = mybir.dt.float32
    bf16 = mybir.dt.bfloat16
    Exp = mybir.ActivationFunctionType.Exp
    Sqrt = mybir.ActivationFunctionType.Sqrt
    mult = mybir.AluOpType.mult

    cpool = ctx.enter_context(tc.tile_pool(name="consts", bufs=1))
    pers = ctx.enter_context(tc.tile_pool(name="persist", bufs=1))

    # ---- constants / weights into SBUF (DMA order = need order) ----
    def cload(name, shape, dt=bf16, eng=None):
        t = cpool.tile(list(shape), dt, tag=name, name=name)
        (eng or nc.sync).dma_start(t[:], io[name][:])
        return t

    # split DGE programming across both hardware queues at startup
    wkv = cload("wkv", (P, 16, P))
    cp = cload("cpack", (P, 528))
    wqa = cload("wqa", (P, 16, P), eng=nc.scalar)
    wqb = cload("wqb", (P, 16, P), eng=nc.scalar)
    tabs = cload("tabs", (P, 2 * S), eng=nc.scalar)
    wo = cload("wo", (P, 2, D), eng=nc.scalar)
    rot2 = cp[:, 0:128]
    tri = cp[:, 128:256]
    identb = cp[:, 256:320]
    sel66 = cp[0:66, 320:448]
    ones2 = cp[:, 448:450]
    onesk = cp[0:HD, 450:451]
    sel1 = cp[0:1, 451:515]
    cos4 = tabs[:, 0:S]
    sin4s = tabs[:, S : 2 * S]

    # ---- persistent activations ----
    QT = [pers.tile([P, S], bf16, tag=f"qt{t}", name=f"QT{t}") for t in range(2)]
    KT = pers.tile([P, S], bf16, tag="kt")  # rows 64-127 = copy of rows 0-63
    V = pers.tile([P, 16, HD + 1], bf16, tag="v")
    OT = pers.tile([P, 2, S], bf16, tag="ot")

    nc.vector.memset(V[:, :, HD : HD + 1], 1.0)
    epsc = pers.tile([P, 1], f32, tag="epsc")
    nc.vector.memset(epsc[:], EPS)

    # ---- pools (PSUM: mmp 2 + opp 2 + pop 2 + dpp 2 = 8 banks) ----
    mmp = ctx.enter_context(tc.tile_pool(name="mmp", bufs=3, space="PSUM"))
    opp = ctx.enter_context(tc.tile_pool(name="opp", bufs=1, space="PSUM"))
    pop = ctx.enter_context(tc.tile_pool(name="pop", bufs=2, space="PSUM"))
    dpp = ctx.enter_context(tc.tile_pool(name="dpp", bufs=2, space="PSUM"))

    xp = ctx.enter_context(tc.tile_pool(name="xp", bufs=3))
    sqp = ctx.enter_context(tc.tile_pool(name="sqp", bufs=2))
    tsp = ctx.enter_context(tc.tile_pool(name="tsp", bufs=3))
    stdp = ctx.enter_context(tc.tile_pool(name="stdp", bufs=2))
    rstdp = ctx.enter_context(tc.tile_pool(name="rstdp", bufs=2))
    rsbp = ctx.enter_context(tc.tile_pool(name="rsbp", bufs=2))
    tcp = ctx.enter_context(tc.tile_pool(name="tcp", bufs=2))
    esp = ctx.enter_context(tc.tile_pool(name="esp", bufs=4))
    recp = ctx.enter_context(tc.tile_pool(name="recp", bufs=2))
    rebp = ctx.enter_context(tc.tile_pool(name="rebp", bufs=2))
    bcsp = ctx.enter_context(tc.tile_pool(name="bcsp", bufs=2))
    stgp = ctx.enter_context(tc.tile_pool(name="stgp", bufs=2))
    ovp = ctx.enter_context(tc.tile_pool(name="ovp", bufs=4))

    # ========== proj + RMSNorm + RoPE for one 512-col chunk, as closures ====
    # Emitted either eagerly or interleaved into the attention stream via the
    # filler deque, so the PE never starves while ACT digests exps.
    def proj_closures(sc):
        cs = slice(sc * 512, (sc + 1) * 512)
        units = []
        u = units.append
        state = {}

        def dma_xc():
            state["xc"] = xp.tile([P, 16, 512], bf16, tag="xc", name="xc")
            if sc == 0:
                # stream chunk 0 in pieces so the first matmuls start early
                for k4 in range(4):
                    nc.sync.dma_start(
                        state["xc"][:, k4 * 4 : (k4 + 1) * 4, :],
                        io["xt"][:, k4 * 4 : (k4 + 1) * 4, cs],
                    )
            else:
                nc.sync.dma_start(state["xc"][:], io["xt"][:, :, cs])
        u(dma_xc)

        def mm_pair(w, pstag, pool, kc):
            def f():
                if pstag not in state:
                    state[pstag] = pool.tile([P, 512], f32, tag=pool.name, name=pstag)
                ps = state[pstag]
                for k in (kc, kc + 1):
                    nc.tensor.matmul(
                        ps, w[:, k, :], state["xc"][:, k, :],
                        start=(k == 0), stop=(k == 15),
                    )
            return f

        def ts_copy(pstag, tstag):
            def f():
                state[tstag] = tsp.tile([P, 512], bf16, tag="ts", name=tstag)
                nc.scalar.copy(state[tstag], state[pstag])
            return f

        def sq_stats(pstag, tstag, m, rowbase, sumsel):
            def f():
                if "stats" not in state:
                    state["stats"] = dpp.tile([P, 512], f32, tag="dp", name="stats")
                    # rows 2:32, 34:64, 65 are never written by the stat
                    # matmuls but are read by the bundled Sqrt; engine
                    # partition bases must be 32-aligned, so define the whole
                    # tile and let the stat matmuls overlay their rows
                    nc.vector.memset(state["stats"][:, :], 1.0)
                sq = sqp.tile([P, 512], bf16, tag="sq", name="sq")[:m]
                nc.vector.tensor_mul(sq, state[pstag][:m], state[tstag][:m])
                nh = 1 if m == HD else 2
                nc.tensor.matmul(
                    state["stats"][rowbase : rowbase + nh], sumsel, sq,
                    start=True, stop=True, skip_group_check=True,
                )
            return f

        # combined K^T|V^T projection: psum rows 0:64 = K^T, 64:128 = V^T
        for kc in range(0, 16, 2):
            u(mm_pair(wkv, "pskv", mmp, kc))
        u(ts_copy("pskv", "tkv"))

        def vtrans(b):
            def f():
                ptf = dpp.tile([P, 512], f32, tag="dp", name="ptf")[:, 0:32]
                pt = ptf.bitcast(bf16)
                nc.tensor.transpose(
                    pt, state["tkv"][HD:P, b * P : (b + 1) * P], identb[HD:P]
                )
                nc.scalar.copy(V[:, sc * 4 + b, 0:HD], pt)
            return f

        for b in range(4):
            u(vtrans(b))
        u(sq_stats("pskv", "tkv", HD, 64, onesk))

        for kc in range(0, 16, 2):
            u(mm_pair(wqa, "ps0", opp, kc))
        u(ts_copy("ps0", "ts0"))
        u(sq_stats("ps0", "ts0", P, 0, ones2))
        for kc in range(0, 16, 2):
            u(mm_pair(wqb, "ps1", mmp, kc))
        u(ts_copy("ps1", "ts1"))
        u(sq_stats("ps1", "ts1", P, 32, ones2))

        def stats_fin():
            # one Sqrt / reciprocal / cast for all 5 head rows of the chunk
            stdall = stdp.tile([66, 512], f32, tag="std", name="stdall")
            nc.scalar.activation(
                stdall, state["stats"][0:66], Sqrt, bias=epsc[:66], scale=1.0 / HD
            )
            rstd = rstdp.tile([66, 512], f32, tag="rstd", name="rstd")
            nc.vector.reciprocal_approx_fast(rstd, stdall)
            state["rsb"] = rsbp.tile([66, 512], bf16, tag="rstdb", name="rsb")
            nc.vector.tensor_copy(state["rsb"], rstd)
        u(stats_fin)

        def bc_tmul(tstag, m, rowbase, T):
            def f():
                bc = dpp.tile([P, 512], f32, tag="dp", name="bc")[:m]
                nh = 1 if m == HD else 2
                nc.tensor.matmul(
                    bc, sel66[rowbase : rowbase + nh, :m],
                    state["rsb"][rowbase : rowbase + nh],
                    start=True, stop=True,
                )
                # T = ts * bcast(g * rstd)  (g folded into sel66 on the host)
                nc.vector.tensor_mul(T, state[tstag][:m], bc)
            return f

        def rope_a(m, T):
            def f():
                state["tmpc"] = tcp.tile([P, 512], bf16, tag="tc", name="tmpc")[:m]
                nc.vector.tensor_mul(state["tmpc"], T, cos4[:m, cs])
                sw = dpp.tile([P, 512], f32, tag="dp", name="sw")[:m]
                nc.tensor.matmul(sw, rot2[:m, :m], T, start=True, stop=True)
                state["sw"] = sw
            return f

        def rope_b(m, T, kdup=False):
            def f():
                nc.vector.tensor_mul(T, state["sw"], sin4s[:m, cs])
                nc.vector.tensor_add(T, T, state["tmpc"])
                if kdup:
                    nc.sync.dma_start(KT[HD:P, cs], KT[0:HD, cs])
            return f

        u(bc_tmul("tkv", HD, 64, KT[0:HD, cs]))
        u(rope_a(HD, KT[0:HD, cs]))
        u(rope_b(HD, KT[0:HD, cs], kdup=True))
        u(bc_tmul("ts0", P, 0, QT[0][:, cs]))
        u(rope_a(P, QT[0][:, cs]))
        u(rope_b(P, QT[0][:, cs]))
        u(bc_tmul("ts1", P, 32, QT[1][:, cs]))
        u(rope_a(P, QT[1][:, cs]))
        u(rope_b(P, QT[1][:, cs]))
        return units

    def outproj_units(qc):
        units = []
        state = {}
        for ms in range(4):
            for dc in range(4):
                def f(ms=ms, dc=dc):
                    sl = slice(qc * 512 + ms * P, qc * 512 + (ms + 1) * P)
                    pso = opp.tile([P, 512], f32, tag="opp", name="pso")
                    for kc in range(2):
                        nc.tensor.matmul(
                            pso, OT[:, kc, sl],
                            wo[:, kc, dc * 512 : (dc + 1) * 512],
                            start=(kc == 0), stop=(kc == 1),
                        )
                    if dc == 0:
                        state[ms] = ovp.tile([P, 4, 512], bf16, tag="ov", name="ov")
                    ov = state[ms]
                    nc.vector.tensor_copy(ov[:, dc, :], pso[:])
                    if dc == 3:
                        nc.sync.dma_start(io["out"][sl, :], ov[:])
                units.append(f)
        return units

    # ================= attention, with filler interleave ===================
    fill = deque()

    def attn_chunk(qc):
        qs = slice(qc * 512, (qc + 1) * 512)
        nkb = 4 * qc + 4
        slots = [4 * nkb]

        def tick():
            k = math.ceil(len(fill) / slots[0]) if fill else 0
            slots[0] -= 1
            for _ in range(min(k, len(fill))):
                fill.popleft()()

        for h in range(4):
            pair, poff = h // 2, (h % 2) * HD
            Q = QT[pair]
            po = pop.tile([HD + 1, 512], f32, tag="po", name="po")

            def score_exp(kb):
                # diagonal blocks (o >= 0): only columns >= 128*o can attend
                # to this key block -> narrow the score/exp/PV to [co:512]
                o = kb - 4 * qc
                co = max(0, o) * P
                ps = mmp.tile([P, 512], f32, tag="mmp", name="ps")
                nc.tensor.matmul(
                    ps[:, co:512],
                    KT[poff : poff + HD, kb * P : (kb + 1) * P],
                    Q[poff : poff + HD, qc * 512 + co : (qc + 1) * 512],
                    start=True, stop=True,
                )
                es = esp.tile([P, 512], bf16, tag="es", name="es")
                nc.scalar.activation(es[:, co:512], ps[:, co:512], Exp, scale=SCALE)
                if o >= 0:
                    # triangular mask on the 128-col diagonal sub-block
                    nc.gpsimd.tensor_mul(es[:, co : co + P], es[:, co : co + P], tri)
                return es, co

            def pv(kb, es, co):
                nc.tensor.matmul(
                    po[:, co:512], V[:, kb, :], es[:, co:512],
                    start=(kb == 0), stop=(kb == nkb - 1),
                    skip_group_check=True,
                )

            prev, pco = score_exp(0)
            for kb in range(1, nkb):
                cur, cco = score_exp(kb)
                pv(kb - 1, prev, pco)
                tick()
                prev, pco = cur, cco
            pv(nkb - 1, prev, pco)
            tick()

            # normalize: row HD of po holds the softmax denominators
            den = recp.tile([1, 512], f32, tag="den", name="den")
            nc.vector.tensor_copy(den, po[HD : HD + 1, :])
            rec = recp.tile([1, 512], f32, tag="rec", name="rec")
            nc.vector.reciprocal_approx_fast(rec, den)
            recb = rebp.tile([1, 512], bf16, tag="recb", name="recb")
            nc.vector.tensor_copy(recb, rec)
            bca = dpp.tile([P, 512], f32, tag="dp", name="bca")[:HD]
            nc.tensor.matmul(bca, sel1, recb, start=True, stop=True)
            bcs = bcsp.tile([HD, 512], bf16, tag="bcs", name="bcs")
            nc.vector.tensor_copy(bcs, bca)
            if poff == 0:
                nc.vector.tensor_mul(OT[0:HD, pair, qs], po[0:HD, :], bcs)
            else:
                stg = stgp.tile([HD, 512], bf16, tag="stg", name="stg")
                nc.vector.tensor_mul(stg, po[0:HD, :], bcs)
                nc.sync.dma_start(OT[HD:P, pair, qs], stg[:])

    for f in proj_closures(0):
        f()
    for f in proj_closures(1):
        f()
    for qc in range(4):
        if qc + 2 <= 3:
            fill.extend(proj_closures(qc + 2))
        attn_chunk(qc)
        fill.extend(outproj_units(qc))
    while fill:
        fill.popleft()()


def _prep_core_inputs(i, x, cos, sin, g_q, g_k, Wq, Wk, Wv, Wo):
    c0 = i * 4 * HD
    k0 = i * HD

    def b(a):
        return np.ascontiguousarray(a).astype(BF)

    x2 = x.reshape(S, D)
    xt = x2.reshape(S, 16, P).transpose(2, 1, 0)  # [p, kc, s]
    wqa = Wq[:, c0 : c0 + P].reshape(16, P, P).transpose(1, 0, 2)
    wqb = Wq[:, c0 + P : c0 + 2 * P].reshape(16, P, P).transpose(1, 0, 2)
    wkv = np.concatenate(
        [Wk[:, k0 : k0 + HD], Wv[:, k0 : k0 + HD]], axis=1
    ).reshape(16, P, P).transpose(1, 0, 2)
    wo = Wo[c0 : c0 + 2 * P, :].reshape(2, P, D).transpose(1, 0, 2)
    cosT = cos.T.astype(np.float32)  # [32, S]
    sinT = sin.T.astype(np.float32)
    cos4 = np.tile(cosT, (4, 1))
    sin4s = np.concatenate([-sinT, sinT, -sinT, sinT], axis=0)
    tabs = np.concatenate([cos4, sin4s], axis=1)  # [128, 4096]
    tri = np.triu(np.ones((P, P), dtype=np.float32))  # [k within blk, q within blk]
    ones2 = np.zeros((P, 2), dtype=np.float32)
    ones2[:HD, 0] = 1.0
    ones2[HD:, 1] = 1.0
    sel66 = np.zeros((66, P), dtype=np.float32)
    for rb in (0, 32):
        sel66[rb, :HD] = g_q
        sel66[rb + 1, HD:] = g_q
    sel66[64, :HD] = g_k
    r64 = np.roll(np.eye(HD, dtype=np.float32), 32, axis=0)
    rot2 = np.zeros((P, P), dtype=np.float32)
    rot2[:HD, :HD] = r64
    rot2[HD:, HD:] = r64
    cpack = np.zeros((P, 528), dtype=np.float32)
    cpack[:, 0:128] = rot2
    cpack[:, 128:256] = tri
    cpack[:, 256:320] = np.concatenate([np.eye(HD), np.eye(HD)], axis=0)
    cpack[0:66, 320:448] = sel66
    cpack[:, 448:450] = ones2
    cpack[0:HD, 450] = 1.0
    cpack[0, 451:515] = 1.0
    return {
        "xt": b(xt),
        "wqa": b(wqa), "wqb": b(wqb), "wkv": b(wkv), "wo": b(wo),
        "tabs": b(tabs), "cpack": b(cpack),
    }


def kernel(x, cos, sin, g_q, g_k, Wq, Wk, Wv, Wo):
    global LAST_RESULTS
    from concourse.bass_utils import run_bass_kernel_spmd

    if "nc" not in _CACHE:
        _CACHE["nc"] = _build_nc()
    nc = _CACHE["nc"]

    args = [np.asarray(a, dtype=np.float32) for a in
            (x, cos, sin, g_q, g_k, Wq, Wk, Wv, Wo)]
    in_maps = [_prep_core_inputs(i, *args) for i in range(N_CORES)]
    trace = bool(os.environ.get("BASS_TRACE"))
    res = run_bass_kernel_spmd(nc, in_maps, list(range(N_CORES)), trace=trace)
    LAST_RESULTS = res
    out = np.zeros((S, D), dtype=np.float32)
    for r in res.results:
        out += np.asarray(r["out"], dtype=np.float32)
    return out.reshape(1, S, D)


# revision 21
# speedup vs baseline: 1.2249x; 1.0433x over previous
"""GQA attention kernel for 8 trn2 NeuronCores (tensor-parallel over heads).

Problem: B=1, S=2048, D=2048, NQ=32 q heads, NKV=8 kv heads, HD=64.
Core i handles q heads 4i..4i+3 and kv head i; out = sum of per-core partials.

v2: all-bf16 matmuls (1 cycle/row vs 4 for fp32 on the PE), x pre-transposed
on the host (kills 256 on-device PE transposes), proj+RMSNorm+RoPE fused per
512-column chunk, ACT stays on the exp table for the whole attention phase,
reciprocals via the fast custom-DVE op, psum->sbuf copies on the Pool engine,
out-projection matmuls interleaved into the attention stream to keep the PE
fed while ACT works through the exps.

Layout (all transposed, zero on-device transposes):
  xT   [128, 16, 2048] bf16  built on host: xT[p, kc, s] = x[s, 128*kc+p]
  Q^T  [128 = 2 heads x 64, S] per head-pair  (lhsT = Wq slice as stored)
  K^T  [64, S] normed+roped, duplicated into partitions 64..127
  V    [128 seq, 16 blocks, 64+1] with a ones column (softmax denominators
       fall out of the PV matmul as row 64)
  S^T block = K^T_slice.T @ Q^T -> exp on ACT -> PV: V_ext.T @ expS^T
  out-proj: lhsT = O^T directly, partial written to DRAM in bf16

RMSNorm over the head dim (= partitions) via ones-selector matmuls; the
per-head g vector is folded into the rstd-broadcast selector on the host.
"""

import os
import sys

sys.path.insert(0, "/opt/trn_rl_repo")

import numpy as np

try:
    import ml_dtypes

    BF = ml_dtypes.bfloat16
except ImportError:  # pragma: no cover
    BF = np.float32

S = 2048
D = 2048
HD = 64
NQ = 32
NKV = 8
P = 128
EPS = 1e-6
SCALE = 0.125  # 1/sqrt(HD)
N_CORES = 8

_CACHE = {}
LAST_RESULTS = None


def _build_nc():
    import concourse.bass as bass
    import concourse.tile as tile
    from concourse import bacc, mybir

    f32 = mybir.dt.float32
    bf16 = mybir.dt.bfloat16
    nc = bacc.Bacc("TRN2", target_bir_lowering=False, debug=False)

    def dram_in(name, shape, dt):
        return nc.dram_tensor(name, list(shape), dt, kind="ExternalInput").ap()

    io = {
        "xt": dram_in("xt", (P, 16, S), bf16),
        "wqa": dram_in("wqa", (P, 16, P), bf16),
        "wqb": dram_in("wqb", (P, 16, P), bf16),
        "wkv": dram_in("wkv", (P, 16, P), bf16),
        "wo": dram_in("wo", (P, 2, D), bf16),
        "tabs": dram_in("tabs", (P, 2 * S), bf16),
        "cpack": dram_in("cpack", (P, 528), bf16),
        "out": nc.dram_tensor("out", [S, D], bf16, kind="ExternalOutput").ap(),
    }

    from contextlib import ExitStack

    with tile.TileContext(nc) as tc, ExitStack() as ctx:
        _emit(ctx, tc, io, bass, mybir)
    nc.compile()
    return nc


def _emit(ctx, tc, io, bass, mybir):
    import math
    from collections import deque

    nc = tc.nc
    f32 = mybir.dt.float32
    bf16 = mybir.dt.bfloat16
    Exp = mybir.ActivationFunctionType.Exp
    Sqrt = mybir.ActivationFunctionType.Sqrt
    mult = mybir.AluOpType.mult

    cpool = ctx.enter_context(tc.tile_pool(name="consts", bufs=1))
    pers = ctx.enter_context(tc.tile_pool(name="persist", bufs=1))

    # ---- constants / weights into SBUF (DMA order = need order) ----
    def cload(name, shape, dt=bf16, eng=None):
        t = cpool.tile(list(shape), dt, tag=name, name=name)
        (eng or nc.sync).dma_start(t[:], io[name][:])
        return t

    # split DGE programming across both hardware queues at startup
    wkv = cload("wkv", (P, 16, P))
    cp = cload("cpack", (P, 528), eng=nc.scalar)
    wqa = cload("wqa", (P, 16, P), eng=nc.scalar)
    wqb = cload("wqb", (P, 16, P), eng=nc.scalar)
    tabs = cload("tabs", (P, 2 * S), eng=nc.scalar)
    wo = cload("wo", (P, 2, D), eng=nc.scalar)
    rot2 = cp[:, 0:128]
    tri = cp[:, 128:256]
    identb = cp[:, 256:320]
    sel66 = cp[0:66, 320:448]
    ones2 = cp[:, 448:450]
    onesk = cp[0:HD, 450:451]
    sel1 = cp[0:1, 451:515]
    cos4 = tabs[:, 0:S]
    sin4s = tabs[:, S : 2 * S]

    # ---- persistent activations ----
    QT = [pers.tile([P, S], bf16, tag=f"qt{t}", name=f"QT{t}") for t in range(2)]
    KT = pers.tile([P, S], bf16, tag="kt")  # rows 64-127 = copy of rows 0-63
    V = pers.tile([P, 16, HD + 1], bf16, tag="v")
    OT = pers.tile([P, 2, S], bf16, tag="ot")

    nc.vector.memset(V[:, :, HD : HD + 1], 1.0)
    epsc = pers.tile([P, 1], f32, tag="epsc")
    nc.vector.memset(epsc[:], EPS)

    # ---- pools (PSUM: mmp 2 + opp 2 + pop 2 + dpp 2 = 8 banks) ----
    mmp = ctx.enter_context(tc.tile_pool(name="mmp", bufs=3, space="PSUM"))
    opp = ctx.enter_context(tc.tile_pool(name="opp", bufs=1, space="PSUM"))
    pop = ctx.enter_context(tc.tile_pool(name="pop", bufs=2, space="PSUM"))
    dpp = ctx.enter_context(tc.tile_pool(name="dpp", bufs=2, space="PSUM"))

    xp = ctx.enter_context(tc.tile_pool(name="xp", bufs=3))
    sqp = ctx.enter_context(tc.tile_pool(name="sqp", bufs=2))
    tsp = ctx.enter_context(tc.tile_pool(name="tsp", bufs=3))
    stdp = ctx.enter_context(tc.tile_pool(name="stdp", bufs=2))
    rstdp = ctx.enter_context(tc.tile_pool(name="rstdp", bufs=2))
    rsbp = ctx.enter_context(tc.tile_pool(name="rsbp", bufs=2))
    tcp = ctx.enter_context(tc.tile_pool(name="tcp", bufs=2))
    esp = ctx.enter_context(tc.tile_pool(name="esp", bufs=4))
    recp = ctx.enter_context(tc.tile_pool(name="recp", bufs=2))
    rebp = ctx.enter_context(tc.tile_pool(name="rebp", bufs=2))
    bcsp = ctx.enter_context(tc.tile_pool(name="bcsp", bufs=2))
    stgp = ctx.enter_context(tc.tile_pool(name="stgp", bufs=2))
    ovp = ctx.enter_context(tc.tile_pool(name="ovp", bufs=4))

    # ========== proj + RMSNorm + RoPE for one 512-col chunk, as closures ====
    # Emitted either eagerly or interleaved into the attention stream via the
    # filler deque, so the PE never starves while ACT digests exps.
    def proj_closures(sc):
        cs = slice(sc * 512, (sc + 1) * 512)
        units = []
        u = units.append
        state = {}

        def dma_xc():
            state["xc"] = xp.tile([P, 16, 512], bf16, tag="xc", name="xc")
            if sc == 0:
                # stream chunk 0 in pieces so the first matmuls start early
                for k4 in range(4):
                    nc.sync.dma_start(
                        state["xc"][:, k4 * 4 : (k4 + 1) * 4, :],
                        io["xt"][:, k4 * 4 : (k4 + 1) * 4, cs],
                    )
            else:
                nc.sync.dma_start(state["xc"][:], io["xt"][:, :, cs])
        u(dma_xc)

        def mm_pair(w, pstag, pool, kc):
            def f():
                if pstag not in state:
                    state[pstag] = pool.tile([P, 512], f32, tag=pool.name, name=pstag)
                ps = state[pstag]
                for k in (kc, kc + 1):
                    nc.tensor.matmul(
                        ps, w[:, k, :], state["xc"][:, k, :],
                        start=(k == 0), stop=(k == 15),
                    )
            return f

        def ts_copy(pstag, tstag):
            def f():
                state[tstag] = tsp.tile([P, 512], bf16, tag="ts", name=tstag)
                nc.scalar.copy(state[tstag], state[pstag])
            return f

        def sq_stats(pstag, tstag, m, rowbase, sumsel):
            def f():
                if "stats" not in state:
                    state["stats"] = dpp.tile([P, 512], f32, tag="dp", name="stats")
                    # rows 2:32, 34:64, 65 are never written by the stat
                    # matmuls but are read by the bundled Sqrt; engine
                    # partition bases must be 32-aligned, so define the whole
                    # tile and let the stat matmuls overlay their rows
                    nc.vector.memset(state["stats"][:, :], 1.0)
                sq = sqp.tile([P, 512], bf16, tag="sq", name="sq")[:m]
                nc.vector.tensor_mul(sq, state[pstag][:m], state[tstag][:m])
                nh = 1 if m == HD else 2
                nc.tensor.matmul(
                    state["stats"][rowbase : rowbase + nh], sumsel, sq,
                    start=True, stop=True, skip_group_check=True,
                )
            return f

        # combined K^T|V^T projection: psum rows 0:64 = K^T, 64:128 = V^T
        for kc in range(0, 16, 2):
            u(mm_pair(wkv, "pskv", mmp, kc))
        u(ts_copy("pskv", "tkv"))

        def vtrans(b):
            def f():
                ptf = dpp.tile([P, 512], f32, tag="dp", name="ptf")[:, 0:32]
                pt = ptf.bitcast(bf16)
                nc.tensor.transpose(
                    pt, state["tkv"][HD:P, b * P : (b + 1) * P], identb[HD:P]
                )
                nc.scalar.copy(V[:, sc * 4 + b, 0:HD], pt)
            return f

        for b in range(4):
            u(vtrans(b))
        u(sq_stats("pskv", "tkv", HD, 64, onesk))

        for kc in range(0, 16, 2):
            u(mm_pair(wqa, "ps0", opp, kc))
        u(ts_copy("ps0", "ts0"))
        u(sq_stats("ps0", "ts0", P, 0, ones2))
        for kc in range(0, 16, 2):
            u(mm_pair(wqb, "ps1", mmp, kc))
        u(ts_copy("ps1", "ts1"))
        u(sq_stats("ps1", "ts1", P, 32, ones2))

        def stats_fin():
            # one Sqrt / reciprocal / cast for all 5 head rows of the chunk
            stdall = stdp.tile([66, 512], f32, tag="std", name="stdall")
            nc.scalar.activation(
                stdall, state["stats"][0:66], Sqrt, bias=epsc[:66], scale=1.0 / HD
            )
            rstd = rstdp.tile([66, 512], f32, tag="rstd", name="rstd")
            nc.vector.reciprocal_approx_fast(rstd, stdall)
            state["rsb"] = rsbp.tile([66, 512], bf16, tag="rstdb", name="rsb")
            nc.vector.tensor_copy(state["rsb"], rstd)
        u(stats_fin)

        def bc_tmul(tstag, m, rowbase, T):
            def f():
                bc = dpp.tile([P, 512], f32, tag="dp", name="bc")[:m]
                nh = 1 if m == HD else 2
                nc.tensor.matmul(
                    bc, sel66[rowbase : rowbase + nh, :m],
                    state["rsb"][rowbase : rowbase + nh],
                    start=True, stop=True,
                )
                # T = ts * bcast(g * rstd)  (g folded into sel66 on the host)
                nc.vector.tensor_mul(T, state[tstag][:m], bc)
            return f

        def rope_a(m, T):
            def f():
                state["tmpc"] = tcp.tile([P, 512], bf16, tag="tc", name="tmpc")[:m]
                nc.vector.tensor_mul(state["tmpc"], T, cos4[:m, cs])
                sw = dpp.tile([P, 512], f32, tag="dp", name="sw")[:m]
                nc.tensor.matmul(sw, rot2[:m, :m], T, start=True, stop=True)
                state["sw"] = sw
            return f

        def rope_b(m, T, kdup=False):
            def f():
                nc.vector.tensor_mul(T, state["sw"], sin4s[:m, cs])
                nc.vector.tensor_add(T, T, state["tmpc"])
                if kdup:
                    nc.sync.dma_start(KT[HD:P, cs], KT[0:HD, cs])
            return f

        u(bc_tmul("tkv", HD, 64, KT[0:HD, cs]))
        u(rope_a(HD, KT[0:HD, cs]))
        u(rope_b(HD, KT[0:HD, cs], kdup=True))
        u(bc_tmul("ts0", P, 0, QT[0][:, cs]))
        u(rope_a(P, QT[0][:, cs]))
        u(rope_b(P, QT[0][:, cs]))
        u(bc_tmul("ts1", P, 32, QT[1][:, cs]))
        u(rope_a(P, QT[1][:, cs]))
        u(rope_b(P, QT[1][:, cs]))
        return units

    def outproj_units(qc, flush=False):
        units = []
        state = {}
        for ms in range(4):
            for dc in range(4):
                def f(ms=ms, dc=dc):
                    sl = slice(qc * 512 + ms * P, qc * 512 + (ms + 1) * P)
                    # during the final flush nothing else competes: use two
                    # psum banks and both copy engines to break serialization
                    if flush and (ms + dc) % 2:
                        pso = dpp.tile([P, 512], f32, tag="dp", name="pso")
                    else:
                        pso = opp.tile([P, 512], f32, tag="opp", name="pso")
                    for kc in range(2):
                        nc.tensor.matmul(
                            pso, OT[:, kc, sl],
                            wo[:, kc, dc * 512 : (dc + 1) * 512],
                            start=(kc == 0), stop=(kc == 1),
                        )
                    if dc == 0:
                        state[ms] = ovp.tile([P, 4, 512], bf16, tag="ov", name="ov")
                    ov = state[ms]
                    if flush and (ms + dc) % 2:
                        nc.scalar.copy(ov[:, dc, :], pso[:])
                    else:
                        nc.vector.tensor_copy(ov[:, dc, :], pso[:])
                    if dc == 3:
                        nc.sync.dma_start(io["out"][sl, :], ov[:])
                units.append(f)
        return units

    # ================= attention, with filler interleave ===================
    fill = deque()

    def attn_chunk(qc):
        qs = slice(qc * 512, (qc + 1) * 512)
        nkb = 4 * qc + 4
        slots = [4 * nkb]

        def tick():
            k = math.ceil(len(fill) / slots[0]) if fill else 0
            slots[0] -= 1
            for _ in range(min(k, len(fill))):
                fill.popleft()()

        for h in range(4):
            pair, poff = h // 2, (h % 2) * HD
            Q = QT[pair]
            po = pop.tile([HD + 1, 512], f32, tag="po", name="po")

            def score_exp(kb):
                # diagonal blocks (o >= 0): only columns >= 128*o can attend
                # to this key block -> narrow the score/exp/PV to [co:512]
                o = kb - 4 * qc
                co = max(0, o) * P
                ps = mmp.tile([P, 512], f32, tag="mmp", name="ps")
                nc.tensor.matmul(
                    ps[:, co:512],
                    KT[poff : poff + HD, kb * P : (kb + 1) * P],
                    Q[poff : poff + HD, qc * 512 + co : (qc + 1) * 512],
                    start=True, stop=True,
                )
                es = esp.tile([P, 512], bf16, tag="es", name="es")
                nc.scalar.activation(es[:, co:512], ps[:, co:512], Exp, scale=SCALE)
                if o >= 0:
                    # triangular mask on the 128-col diagonal sub-block
                    nc.gpsimd.tensor_mul(es[:, co : co + P], es[:, co : co + P], tri)
                return es, co

            def pv(kb, es, co):
                nc.tensor.matmul(
                    po[:, co:512], V[:, kb, :], es[:, co:512],
                    start=(kb == 0), stop=(kb == nkb - 1),
                    skip_group_check=True,
                )

            prev, pco = score_exp(0)
            for kb in range(1, nkb):
                cur, cco = score_exp(kb)
                pv(kb - 1, prev, pco)
                tick()
                prev, pco = cur, cco
            pv(nkb - 1, prev, pco)
            tick()

            # normalize: row HD of po holds the softmax denominators
            den = recp.tile([1, 512], f32, tag="den", name="den")
            nc.vector.tensor_copy(den, po[HD : HD + 1, :])
            rec = recp.tile([1, 512], f32, tag="rec", name="rec")
            nc.vector.reciprocal_approx_fast(rec, den)
            recb = rebp.tile([1, 512], bf16, tag="recb", name="recb")
            nc.vector.tensor_copy(recb, rec)
            bca = dpp.tile([P, 512], f32, tag="dp", name="bca")[:HD]
            nc.tensor.matmul(bca, sel1, recb, start=True, stop=True)
            bcs = bcsp.tile([HD, 512], bf16, tag="bcs", name="bcs")
            nc.vector.tensor_copy(bcs, bca)
            if poff == 0:
                nc.vector.tensor_mul(OT[0:HD, pair, qs], po[0:HD, :], bcs)
            else:
                stg = stgp.tile([HD, 512], bf16, tag="stg", name="stg")
                nc.vector.tensor_mul(stg, po[0:HD, :], bcs)
                nc.sync.dma_start(OT[HD:P, pair, qs], stg[:])

    for f in proj_closures(0):
        f()
    for f in proj_closures(1):
        f()
    for qc in range(4):
        if qc + 2 <= 3:
            fill.extend(proj_closures(qc + 2))
        attn_chunk(qc)
        fill.extend(outproj_units(qc, flush=(qc == 3)))
    while fill:
        fill.popleft()()


def _prep_core_inputs(i, x, cos, sin, g_q, g_k, Wq, Wk, Wv, Wo):
    c0 = i * 4 * HD
    k0 = i * HD

    def b(a):
        return np.ascontiguousarray(a).astype(BF)

    x2 = x.reshape(S, D)
    xt = x2.reshape(S, 16, P).transpose(2, 1, 0)  # [p, kc, s]
    wqa = Wq[:, c0 : c0 + P].reshape(16, P, P).transpose(1, 0, 2)
    wqb = Wq[:, c0 + P : c0 + 2 * P].reshape(16, P, P).transpose(1, 0, 2)
    wkv = np.concatenate(
        [Wk[:, k0 : k0 + HD], Wv[:, k0 : k0 + HD]], axis=1
    ).reshape(16, P, P).transpose(1, 0, 2)
    wo = Wo[c0 : c0 + 2 * P, :].reshape(2, P, D).transpose(1, 0, 2)
    cosT = cos.T.astype(np.float32)  # [32, S]
    sinT = sin.T.astype(np.float32)
    cos4 = np.tile(cosT, (4, 1))
    sin4s = np.concatenate([-sinT, sinT, -sinT, sinT], axis=0)
    tabs = np.concatenate([cos4, sin4s], axis=1)  # [128, 4096]
    tri = np.triu(np.ones((P, P), dtype=np.float32))  # [k within blk, q within blk]
    ones2 = np.zeros((P, 2), dtype=np.float32)
    ones2[:HD, 0] = 1.0
    ones2[HD:, 1] = 1.0
    sel66 = np.zeros((66, P), dtype=np.float32)
    for rb in (0, 32):
        sel66[rb, :HD] = g_q
        sel66[rb + 1, HD:] = g_q
    sel66[64, :HD] = g_k
    r64 = np.roll(np.eye(HD, dtype=np.float32), 32, axis=0)
    rot2 = np.zeros((P, P), dtype=np.float32)
    rot2[:HD, :HD] = r64
    rot2[HD:, HD:] = r64
    cpack = np.zeros((P, 528), dtype=np.float32)
    cpack[:, 0:128] = rot2
    cpack[:, 128:256] = tri
    cpack[:, 256:320] = np.concatenate([np.eye(HD), np.eye(HD)], axis=0)
    cpack[0:66, 320:448] = sel66
    cpack[:, 448:450] = ones2
    cpack[0:HD, 450] = 1.0
    cpack[0, 451:515] = 1.0
    return {
        "xt": b(xt),
        "wqa": b(wqa), "wqb": b(wqb), "wkv": b(wkv), "wo": b(wo),
        "tabs": b(tabs), "cpack": b(cpack),
    }


def kernel(x, cos, sin, g_q, g_k, Wq, Wk, Wv, Wo):
    global LAST_RESULTS
    from concourse.bass_utils import run_bass_kernel_spmd

    if "nc" not in _CACHE:
        _CACHE["nc"] = _build_nc()
    nc = _CACHE["nc"]

    args = [np.asarray(a, dtype=np.float32) for a in
            (x, cos, sin, g_q, g_k, Wq, Wk, Wv, Wo)]
    in_maps = [_prep_core_inputs(i, *args) for i in range(N_CORES)]
    trace = bool(os.environ.get("BASS_TRACE"))
    res = run_bass_kernel_spmd(nc, in_maps, list(range(N_CORES)), trace=trace)
    LAST_RESULTS = res
    out = np.zeros((S, D), dtype=np.float32)
    for r in res.results:
        out += np.asarray(r["out"], dtype=np.float32)
    return out.reshape(1, S, D)
